# revision 17
# baseline (speedup 1.0000x reference)
"""Born-Wolf PSF kernel for Trainium2, 8 NeuronCores, data-parallel over batch.

Self-contained: hardcodes all geometry from the problem spec.
  input : params (16, 64, 2) float32
  output: psf    (16, 64, 25, 25, 25) float32

Per (b,c) pair: psf = |trapz_rho J0(k n r rho) exp(-i 0.5 k rho^2 z n^2) rho|^2,
bilinearly interpolated from 35 anchor radii onto a 25x25 grid, reflect-padded
in z, and normalized.

v2 strategy: all per-(rho, pair*anchor) field quantities are sums of outer
products (rank-k separable), so they are computed by TensorE matmuls directly
into PSUM instead of DRAM-row broadcast DMAs (which serialized ~375us on one
DMA engine in v1). f32r matmuls truncate inputs to ~12 mantissa bits, so the
large phase factors (up to ~145 turns) are split hi/lo: hi holds 10 explicit
bits (exact in f32r), lo carries the remainder; the rank-5 expansion
(ones, hi*hi, hi*lo, lo*hi, lo*lo) restores full fp32 accuracy at 1 cycle/col.
Trapezoid weights and 1/2pi factors are folded into the matmul lhs constants;
normalization is folded into the per-partition scale of the PSUM->SBUF copy
after the G-expansion matmul.
"""
import os
import numpy as np

# ---------------- problem geometry (hardcoded) ----------------
B, CH = 16, 64
NCORES = 8
NP = (B // NCORES) * CH          # 128 pairs per core
NA, NJ, NZH, NZ = 35, 101, 13, 25
F = NP * NA                      # 4480
FCH = NP * NZH                   # 1664
NYX = 625
GW = 640                         # zero-padded G columns (2 x 320 matmuls)
PI = float(np.pi)
C0 = -0.1562499995e-1
C1 = -0.1098628627e-2
C2 = 0.1430488765e-3
S_AMP = float(np.sqrt(0.636619772))
SC1 = 20.0                       # power-row scale split (clamp = SC1^2 = 400)
QS = [1.0, -0.25, 0.015624999996, -0.00043402777473, 6.7816828549e-06,
      -6.781657507e-08, 4.7091319698e-10, -2.3995591574e-12,
      9.2118377553e-15, -2.3695100804e-17]
MAGIC = 12582912.0               # 1.5 * 2**23: (t+M)-M == round-to-nearest(t)
TMASK = 4.0 / (2.0 * PI) + 0.125  # mask threshold in turn units
VC = 8193.0                      # Veltkamp split const (2^13+1): hi keeps 10 bits

_CACHE = {}


def _split10(x):
    """Split f32 values into (hi, lo); hi has <=10 explicit mantissa bits."""
    xf = np.ascontiguousarray(np.asarray(x, np.float32))
    hi = (xf.view(np.uint32) & np.uint32(0xFFFFE000)).view(np.float32).copy()
    lo = (xf - hi).astype(np.float32)
    return hi, lo


def _host_consts():
    if "consts" in _CACHE:
        return _CACHE["consts"]
    f32 = np.float32
    R = (np.linspace(0, 34, NA) / 2.0).astype(np.float64)          # anchor radii
    RHO = np.linspace(0.0, 1.0, NJ).astype(np.float64)
    yp = xp = 12.0
    Y, X = np.meshgrid(np.arange(25.0), np.arange(25.0), indexing="ij")
    rPix = np.sqrt((X - xp) ** 2 + (Y - yp) ** 2)
    IDX1 = np.floor(rPix * 2).astype(np.int32)
    IDX2 = IDX1 + 1
    DISR1 = ((rPix - R[IDX1]) * 2).astype(np.float32).astype(np.float64)
    DISR2 = 1.0 - DISR1

    G = np.zeros((NA, GW), np.float64)
    for yy in range(25):
        for xx in range(25):
            yx = yy * 25 + xx
            G[IDX2[yy, xx], yx] += DISR1[yy, xx]
            G[IDX1[yy, xx], yx] += DISR2[yy, xx]
    gcol = G[:, :NYX].sum(1)
    w13 = np.concatenate([[1.0], np.full(NZH - 1, 2.0)])
    gw13 = np.zeros((NA, 2 * NZH))
    gw13[:, 0::2] = gcol[:, None] * w13[None, :]

    wt = np.full(NJ, 0.01, np.float64)
    wt[0] *= 0.5
    wt[-1] *= 0.5
    rw = RHO * wt                                    # trapezoid weight * rho

    with np.errstate(divide="ignore"):
        rinv = 1.0 / RHO
    rinv[0] = 0.0

    # phase lhs: t0 = 0.125 + RHO*knr_t + ps1*rinv/(2pi)^2 + ps3*rinv^3/(2pi)^4
    rho_hi, rho_lo = _split10(RHO)
    L_ph1 = np.stack([np.full(NJ, 0.125), rho_hi, rho_hi, rho_lo, rho_lo])
    L_ph2 = np.stack([rinv / (2 * np.pi) ** 2, rinv ** 3 / (2 * np.pi) ** 4])

    # poly lhs (J0 small branch, trapezoid weight folded in); row 0 <-> ones
    qrho = np.stack([QS[m] * (SC1 * SC1 * RHO ** 2) ** m for m in range(10)])
    L_poly = qrho * rw[None, :]

    # amplitude lhs (J0 large branch, trapezoid weight folded in)
    L_amp = np.stack([np.sqrt(rinv) * rw, -rinv ** 2.5 * rw])

    # C-matrix lhs: turns = RHO^2 * wcz_t  (+0.25 for cos; negated for sin)
    r2hi, r2lo = _split10(RHO ** 2)
    L_cos = np.stack([np.full(NJ, 0.25), r2hi, r2hi, r2lo, r2lo])
    L_sin = np.stack([np.zeros(NJ), -r2hi, -r2hi, -r2lo, -r2lo])

    Rinv = 1.0 / np.maximum(R, 1e-9)
    Rinv[0] = 0.0
    rtab = np.tile(R[None, :], (NP, 1))
    ri8c0 = np.tile((8.0 * C0 * Rinv)[None, :], (NP, 1))
    ri3c2 = np.tile((512.0 * C2 * Rinv ** 3)[None, :], (NP, 1))
    ztab = np.tile(np.arange(NZH, dtype=np.float64)[None, :], (NP, 1))

    consts = {
        "Lph1": L_ph1.astype(f32),
        "Lph2": L_ph2.astype(f32),
        "Lpoly": L_poly.astype(f32),
        "Lamp": L_amp.astype(f32),
        "Lcos": L_cos.astype(f32),
        "Lsin": L_sin.astype(f32),
        "rtab": rtab.astype(f32),
        "ri8c0": ri8c0.astype(f32),
        "ri3c2": ri3c2.astype(f32),
        "ztab": ztab.astype(f32),
        "gpad": G.astype(f32),
        "gw13": gw13.astype(f32),
    }
    for k, v in consts.items():
        assert np.isfinite(v).all(), k
    _CACHE["consts"] = consts
    return consts


def _ensure_paths():
    import sys
    for p in ("/opt/trn_rl_repo", "/root/.axon_site/_ro/trn_rl_repo"):
        if os.path.isdir(p) and p not in sys.path:
            sys.path.append(p)


def _build_nc():
    if "nc" in _CACHE:
        return _CACHE["nc"]
    _ensure_paths()
    from contextlib import ExitStack
    import concourse.bass as bass
    import concourse.bacc as bacc
    import concourse.tile as tile
    from concourse import mybir

    f32 = mybir.dt.float32
    f32r = mybir.dt.float32r
    bf16 = mybir.dt.bfloat16
    u8 = mybir.dt.uint8
    AF = mybir.ActivationFunctionType
    OP = mybir.AluOpType

    nc = bacc.Bacc()
    BIAS_A1 = float(np.log(S_AMP) - 0.5 * np.log(2 * np.pi))
    BIAS_A2 = float(np.log(64.0 * abs(C1) * S_AMP) - 2.5 * np.log(2 * np.pi))
    for val in (BIAS_A1, BIAS_A2):
        t = nc.alloc_sbuf_tensor(f"const-f32-{val}", [128, 1], f32)
        nc.gpsimd.memset(t.ap(), val)
        nc.const_aps.aps[(f32, val)] = t.ap()
    nc.all_engine_barrier()

    d_par = nc.declare_dram_parameter("params", [NP, 2], f32, isOutput=False)
    d_lph1 = nc.declare_dram_parameter("Lph1", [5, NJ], f32, isOutput=False)
    d_lph2 = nc.declare_dram_parameter("Lph2", [2, NJ], f32, isOutput=False)
    d_lpoly = nc.declare_dram_parameter("Lpoly", [10, NJ], f32, isOutput=False)
    d_lamp = nc.declare_dram_parameter("Lamp", [2, NJ], f32, isOutput=False)
    d_lcos = nc.declare_dram_parameter("Lcos", [5, NJ], f32, isOutput=False)
    d_lsin = nc.declare_dram_parameter("Lsin", [5, NJ], f32, isOutput=False)
    d_rtab = nc.declare_dram_parameter("rtab", [NP, NA], f32, isOutput=False)
    d_ri1 = nc.declare_dram_parameter("ri8c0", [NP, NA], f32, isOutput=False)
    d_ri3 = nc.declare_dram_parameter("ri3c2", [NP, NA], f32, isOutput=False)
    d_z = nc.declare_dram_parameter("ztab", [NP, NZH], f32, isOutput=False)
    d_g = nc.declare_dram_parameter("gpad", [NA, GW], f32, isOutput=False)
    d_gw = nc.declare_dram_parameter("gw13", [NA, 2 * NZH], f32,
                                     isOutput=False)
    d_out = nc.declare_dram_parameter("out", [NP, NZ, NYX], f32, isOutput=True)

    with tile.TileContext(nc) as tc, ExitStack() as ctx:
        p1 = ctx.enter_context(tc.tile_pool(name="p1", bufs=1))
        p2 = ctx.enter_context(tc.tile_pool(name="p2", bufs=3))

        # ---- const loads ----
        t_par = p1.tile([NP, 2], f32, tag="par")
        t_lph1f = p1.tile([5, NJ], f32, tag="lph1f")
        t_lph2f = p1.tile([2, NJ], f32, tag="lph2f")
        t_lpolyf = p1.tile([10, NJ], f32, tag="lpolyf")
        t_lampf = p1.tile([2, NJ], f32, tag="lampf")
        t_lcosf = p1.tile([5, NJ], f32, tag="lcosf")
        t_lsinf = p1.tile([5, NJ], f32, tag="lsinf")
        t_rtab = p1.tile([NP, NA], f32, tag="rtab")
        t_ri1 = p1.tile([NP, NA], f32, tag="ri1")
        t_ri3 = p1.tile([NP, NA], f32, tag="ri3")
        t_z = p1.tile([NP, NZH], f32, tag="ztab")
        t_gf = p1.tile([NA, GW], f32, tag="gpadf")
        t_gwf = p1.tile([NA, 2 * NZH], f32, tag="gw13f")
        for t, d in ((t_par, d_par), (t_lph1f, d_lph1), (t_lph2f, d_lph2),
                     (t_lpolyf, d_lpoly), (t_lampf, d_lamp), (t_lcosf, d_lcos),
                     (t_lsinf, d_lsin), (t_rtab, d_rtab), (t_ri1, d_ri1),
                     (t_ri3, d_ri3), (t_z, d_z), (t_gf, d_g), (t_gwf, d_gw)):
            nc.sync.dma_start(out=t[:], in_=d[:])
        t_lph1 = p1.tile([5, NJ], f32r, tag="lph1")
        t_lph2 = p1.tile([2, NJ], f32r, tag="lph2")
        t_lpoly = p1.tile([10, NJ], f32r, tag="lpoly")
        t_lamp = p1.tile([2, NJ], f32r, tag="lamp")
        t_lcos = p1.tile([5, NJ], f32r, tag="lcos")
        t_lsin = p1.tile([5, NJ], f32r, tag="lsin")
        t_g = p1.tile([NA, GW], f32r, tag="gpad")
        t_gw = p1.tile([NA, 2 * NZH], f32r, tag="gw13")
        for dst, srcf in ((t_lph1, t_lph1f), (t_lph2, t_lph2f),
                          (t_lpoly, t_lpolyf), (t_lamp, t_lampf),
                          (t_lcos, t_lcosf), (t_lsin, t_lsinf),
                          (t_g, t_gf), (t_gw, t_gwf)):
            nc.vector.tensor_copy(dst[:], srcf[:])

        # ---- pair-scalar stage ([NP,1] / [NP,NA]) ----
        t_abs = p1.tile([NP, 2], f32, tag="pabs")
        nc.vector.scalar_tensor_tensor(t_abs[:], t_par[:], -1.0, t_par[:],
                                       OP.mult, OP.max)
        lam = t_abs[:, 0:1]
        enn = t_abs[:, 1:2]
        t_rl = p1.tile([NP, 1], f32, tag="rl")
        nc.vector.reciprocal(t_rl[:], lam)
        t_knt = p1.tile([NP, 1], f32, tag="knt")       # n/lam (turns per R*rho)
        nc.vector.tensor_tensor(t_knt[:], enn, t_rl[:], OP.mult)
        t_rkn = p1.tile([NP, 1], f32, tag="rkn")       # lam/n
        nc.vector.reciprocal(t_rkn[:], t_knt[:])
        t_rkn3 = p1.tile([NP, 1], f32, tag="rkn3")
        nc.vector.tensor_tensor(t_rkn3[:], t_rkn[:], t_rkn[:], OP.mult)
        nc.vector.tensor_tensor(t_rkn3[:], t_rkn3[:], t_rkn[:], OP.mult)
        t_wct = p1.tile([NP, 1], f32, tag="wct")       # 0.5*n^2/lam
        nc.vector.scalar_tensor_tensor(t_wct[:], enn, 0.5, t_knt[:],
                                       OP.mult, OP.mult)

        t_knr = p1.tile([NP, NA], f32, tag="knr")      # knr in turns, <=145
        nc.vector.tensor_scalar(t_knr[:], t_rtab[:], t_knt[:], None, OP.mult)
        # Veltkamp split: hi keeps ~10 bits (exact under f32r truncation)
        t_kv = p1.tile([NP, NA], f32, tag="kv")
        nc.vector.tensor_scalar(t_kv[:], t_knr[:], VC, None, OP.mult)
        t_kz = p1.tile([NP, NA], f32, tag="kz")
        nc.vector.tensor_tensor(t_kz[:], t_kv[:], t_knr[:], OP.subtract)
        t_khi = p1.tile([NP, NA], f32, tag="khi")
        nc.vector.tensor_tensor(t_khi[:], t_kv[:], t_kz[:], OP.subtract)
        t_klo = p1.tile([NP, NA], f32, tag="klo")
        nc.vector.tensor_tensor(t_klo[:], t_knr[:], t_khi[:], OP.subtract)

        t_ps1 = p1.tile([NP, NA], f32, tag="ps1")
        nc.vector.tensor_scalar(t_ps1[:], t_ri1[:], t_rkn[:], None, OP.mult)
        t_ps3 = p1.tile([NP, NA], f32, tag="ps3")
        nc.vector.tensor_scalar(t_ps3[:], t_ri3[:], t_rkn3[:], None, OP.mult)

        t_knm = p1.tile([NP, NA], f32, tag="knm")
        nc.vector.tensor_scalar_max(t_knm[:], t_knr[:], 1e-4)
        t_lk = p1.tile([NP, NA], f32, tag="lk")
        nc.scalar.activation(t_lk[:], t_knm[:], AF.Ln)
        t_a1 = p1.tile([NP, NA], f32, tag="a1")
        nc.scalar.activation(t_a1[:], t_lk[:], AF.Exp, bias=BIAS_A1, scale=-0.5)
        t_a2 = p1.tile([NP, NA], f32, tag="a2")
        nc.scalar.activation(t_a2[:], t_lk[:], AF.Exp, bias=BIAS_A2, scale=-2.5)

        # power rows: v = min(knr_rad/SC1, SC1)^2 ; U[:, m*NA:(m+1)*NA] = v^(m+1)
        t_v0 = p1.tile([NP, NA], f32, tag="v0")
        nc.vector.tensor_scalar(t_v0[:], t_knr[:], 2.0 * PI / SC1, SC1,
                                OP.mult, OP.min)
        t_U = p1.tile([NP, 9 * NA], f32, tag="U")
        nc.vector.tensor_tensor(t_U[:, 0:NA], t_v0[:], t_v0[:], OP.mult)
        for m in range(1, 9):
            nc.vector.tensor_tensor(t_U[:, m * NA:(m + 1) * NA],
                                    t_U[:, (m - 1) * NA:m * NA],
                                    t_U[:, 0:NA], OP.mult)

        # wcz in turns (<=87), Veltkamp split
        t_wcz = p1.tile([NP, NZH], f32, tag="wcz")
        nc.vector.tensor_scalar(t_wcz[:], t_z[:], t_wct[:], None, OP.mult)
        t_wv = p1.tile([NP, NZH], f32, tag="wv")
        nc.vector.tensor_scalar(t_wv[:], t_wcz[:], VC, None, OP.mult)
        t_wz2 = p1.tile([NP, NZH], f32, tag="wz2")
        nc.vector.tensor_tensor(t_wz2[:], t_wv[:], t_wcz[:], OP.subtract)
        t_whi = p1.tile([NP, NZH], f32, tag="whi")
        nc.vector.tensor_tensor(t_whi[:], t_wv[:], t_wz2[:], OP.subtract)
        t_wlo = p1.tile([NP, NZH], f32, tag="wlo")
        nc.vector.tensor_tensor(t_wlo[:], t_wcz[:], t_whi[:], OP.subtract)

        # ---- flatten rows into matmul rhs tiles (SBUF->SBUF DMA) ----
        # sources rounded to f32r first; each rhs tile starts at partition 0
        t_Ur = p1.tile([NP, 9 * NA], f32r, tag="Ur")
        nc.vector.tensor_copy(t_Ur[:], t_U[:])
        t_khir = p1.tile([NP, NA], f32r, tag="khir")
        nc.vector.tensor_copy(t_khir[:], t_khi[:])
        t_klor = p1.tile([NP, NA], f32r, tag="klor")
        nc.vector.tensor_copy(t_klor[:], t_klo[:])
        t_ps1r = p1.tile([NP, NA], f32r, tag="ps1r")
        nc.vector.tensor_copy(t_ps1r[:], t_ps1[:])
        t_ps3r = p1.tile([NP, NA], f32r, tag="ps3r")
        nc.vector.tensor_copy(t_ps3r[:], t_ps3[:])
        t_a1r = p1.tile([NP, NA], f32r, tag="a1r")
        nc.vector.tensor_copy(t_a1r[:], t_a1[:])
        t_a2r = p1.tile([NP, NA], f32r, tag="a2r")
        nc.vector.tensor_copy(t_a2r[:], t_a2[:])
        t_1f = p1.tile([NP, NA], f32, tag="onesf")
        nc.vector.memset(t_1f[:], 1.0)
        t_1r = p1.tile([NP, NA], f32r, tag="onesr")
        nc.vector.tensor_copy(t_1r[:], t_1f[:])
        rowsH = p1.tile([5, F], f32r, tag="rowsH")    # ones | khi klo khi klo
        nc.sync.dma_start(out=rowsH[0:1, :], in_=t_1r[:])
        nc.sync.dma_start(out=rowsH[1:2, :], in_=t_khir[:])
        nc.sync.dma_start(out=rowsH[2:3, :], in_=t_klor[:])
        nc.sync.dma_start(out=rowsH[3:4, :], in_=rowsH[1:2, :])
        nc.sync.dma_start(out=rowsH[4:5, :], in_=rowsH[2:3, :])
        rowsS = p1.tile([2, F], f32r, tag="rowsS")    # ps1 | ps3
        nc.sync.dma_start(out=rowsS[0:1, :], in_=t_ps1r[:])
        nc.sync.dma_start(out=rowsS[1:2, :], in_=t_ps3r[:])
        rowsA = p1.tile([2, F], f32r, tag="rowsA")    # a1 | a2
        nc.sync.dma_start(out=rowsA[0:1, :], in_=t_a1r[:])
        nc.sync.dma_start(out=rowsA[1:2, :], in_=t_a2r[:])
        rowsP = p1.tile([10, F], f32r, tag="rowsP")   # ones | v^1..v^9
        nc.scalar.dma_start(out=rowsP[0:1, :], in_=t_1r[:])
        for m in range(9):
            nc.scalar.dma_start(out=rowsP[m + 1:m + 2, :],
                                in_=t_Ur[:, m * NA:(m + 1) * NA])

        # rowsC rows: 0 ones | 1 whi | 2 wlo | 3 whi | 4 wlo
        t_whir = p1.tile([NP, NZH], f32r, tag="whir")
        nc.vector.tensor_copy(t_whir[:], t_whi[:])
        t_wlor = p1.tile([NP, NZH], f32r, tag="wlor")
        nc.vector.tensor_copy(t_wlor[:], t_wlo[:])
        rowsC = p1.tile([5, FCH], f32r, tag="rowsC")
        nc.scalar.dma_start(out=rowsC[0:1, :], in_=t_1r[:, 0:NZH])
        nc.scalar.dma_start(out=rowsC[1:2, :], in_=t_whir[:])
        nc.scalar.dma_start(out=rowsC[2:3, :], in_=t_wlor[:])
        nc.scalar.dma_start(out=rowsC[3:4, :], in_=rowsC[1:2, :])
        nc.scalar.dma_start(out=rowsC[4:5, :], in_=rowsC[2:3, :])

        # ---- field stage: per-chunk matmuls into PSUM + pointwise ----
        tJ0 = p1.tile([NJ, F], bf16, tag="J0")
        tMask = p1.tile([NJ, F], u8, tag="mask")
        tCT = p1.tile([NJ, NP * 26], bf16, tag="CT")
        ct3 = tCT[:].rearrange("p (n c) -> p n c", c=26)

        CW = 1024
        with tc.tile_pool(name="pf", bufs=2, space="PSUM") as pf:
            # ---- C matrices: cos/sin(2pi * rho^2 * wcz) -> CT bf16 ----
            nb = [0, 78, NP]
            for ci in range(2):
                n0, n1 = nb[ci], nb[ci + 1]
                w = (n1 - n0) * NZH
                slc = slice(n0 * NZH, n0 * NZH + w)
                chws = [(0, min(512, w))] + ([(512, w - 512)] if w > 512
                                              else [])
                for lhs, zoff, ptag in ((t_lcos, 0, "T0"), (t_lsin, NZH, "X")):
                    psCC = pf.tile([NJ, CW], f32, tag=ptag)
                    for o, hw in chws:
                        nc.tensor.matmul(
                            psCC[:, o:o + hw], lhs[:],
                            rowsC[:, n0 * NZH + o:n0 * NZH + o + hw],
                            start=True, stop=True)
                    tCRm = p2.tile([NJ, CW], f32, tag="RRm")
                    nc.scalar.activation(tCRm[:, 0:w], psCC[:, 0:w], AF.Copy,
                                         bias=MAGIC)
                    tCRR = p2.tile([NJ, CW], f32, tag="RR")
                    nc.scalar.activation(tCRR[:, 0:w], tCRm[:, 0:w], AF.Copy,
                                         bias=-MAGIC)
                    tCNU = p2.tile([NJ, CW], f32, tag="NU")
                    nc.vector.tensor_tensor(tCNU[:, 0:w], psCC[:, 0:w],
                                            tCRR[:, 0:w], OP.subtract)
                    nc.scalar.activation(
                        ct3[:, n0:n1, zoff:zoff + NZH],
                        tCNU[:, 0:w].rearrange("p (n z) -> p n z", z=NZH),
                        AF.Sin, scale=2.0 * PI)

            nchunks = (F + CW - 1) // CW
            for c in range(nchunks):
                w = min(CW, F - c * CW)
                sl = slice(c * CW, c * CW + w)
                hws = [(0, min(512, w))] + ([(512, w - 512)] if w > 512
                                             else [])
                psT0 = pf.tile([NJ, CW], f32, tag="T0")
                for o, hw in hws:
                    nc.tensor.matmul(psT0[:, o:o + hw], t_lph1[:],
                                     rowsH[:, c * CW + o:c * CW + o + hw],
                                     start=True, stop=False)
                # mask from the pure x-part (before asymptotic corrections)
                nc.vector.tensor_scalar(tMask[:, sl], psT0[:, 0:w], TMASK,
                                        None, OP.is_le)
                for o, hw in hws:
                    nc.tensor.matmul(psT0[:, o:o + hw], t_lph2[:],
                                     rowsS[:, c * CW + o:c * CW + o + hw],
                                     start=False, stop=True)
                tRRm = p2.tile([NJ, CW], f32, tag="RRm")
                nc.scalar.activation(tRRm[:, 0:w], psT0[:, 0:w], AF.Copy,
                                     bias=MAGIC)
                tRR = p2.tile([NJ, CW], f32, tag="RR")
                nc.scalar.activation(tRR[:, 0:w], tRRm[:, 0:w], AF.Copy,
                                     bias=-MAGIC)
                tNU = p2.tile([NJ, CW], f32, tag="NU")
                nc.vector.tensor_tensor(tNU[:, 0:w], psT0[:, 0:w],
                                        tRR[:, 0:w], OP.subtract)
                tCOS = p2.tile([NJ, CW], f32, tag="COS")
                nc.scalar.activation(tCOS[:, 0:w], tNU[:, 0:w], AF.Sin,
                                     scale=2.0 * PI)
                psX = pf.tile([NJ, CW], f32, tag="X")
                for o, hw in hws:
                    nc.tensor.matmul(psX[:, o:o + hw], t_lamp[:],
                                     rowsA[:, c * CW + o:c * CW + o + hw],
                                     start=True, stop=True)
                nc.vector.tensor_tensor(tJ0[:, sl], psX[:, 0:w],
                                        tCOS[:, 0:w], OP.mult)
                for o, hw in hws:
                    nc.tensor.matmul(psX[:, o:o + hw], t_lpoly[:],
                                     rowsP[:, c * CW + o:c * CW + o + hw],
                                     start=True, stop=True)
                nc.vector.copy_predicated(tJ0[:, sl], tMask[:, sl],
                                          psX[:, 0:w])

        # ---- per-pair contraction into PSUM, 4 waves of 32 pairs ----
        tUS = p1.tile([NA, FCH], f32, tag="plU")
        tVS = p1.tile([NA, FCH], f32, tag="plV")
        tPL = p1.tile([NA, FCH], f32r, tag="plP")
        us_z = tUS[:].rearrange("q (zz pp) -> q pp zz", pp=NP)
        vs_z = tVS[:].rearrange("q (zz pp) -> q pp zz", pp=NP)
        WP = 32
        with tc.tile_pool(name="ppr", bufs=2, space="PSUM") as ppr, \
                tc.tile_pool(name="pso", bufs=1, space="PSUM") as pso, \
                tc.tile_pool(name="pgo", bufs=2, space="PSUM") as pgo:
            for wv in range(NP // WP):
                tPRw = ppr.tile([NA, WP * 32], f32, tag="PR")
                for j in range(WP):
                    p = wv * WP + j
                    nc.tensor.matmul(tPRw[:, j * 32:j * 32 + 26],
                                     tJ0[:, p * NA:(p + 1) * NA],
                                     tCT[:, p * 26:(p + 1) * 26],
                                     start=True, stop=True)
                pr4 = tPRw[:].rearrange("q (n s) -> q n s", s=32)
                slw = slice(wv * WP, (wv + 1) * WP)
                nc.scalar.activation(us_z[:, slw, :], pr4[:, :, 0:NZH],
                                     AF.Square)
                nc.scalar.activation(vs_z[:, slw, :], pr4[:, :, NZH:26],
                                     AF.Square)
            nc.vector.tensor_tensor(tPL[:], tUS[:], tVS[:], OP.add)

            # ---- normalization: nrm[p] = sum_zz sum_a gw13[a,zz]*PL[a,zz*NP+p]
            psN = pso.tile([NP, 2], f32, tag="N")
            for zz in range(NZH):
                nc.tensor.matmul(psN[:], tPL[:, zz * NP:(zz + 1) * NP],
                                 t_gw[:, 2 * zz:2 * zz + 2],
                                 start=(zz == 0), stop=(zz == NZH - 1))
            tRC = p1.tile([NP, 1], f32, tag="RC")
            nc.vector.reciprocal(tRC[:], psN[:, 0:1])

            # ---- G expansion + normalize-on-copy + mirrored output ----
            for zz in range(NZH):
                lhs = tPL[:, zz * NP:(zz + 1) * NP]
                tOS = p2.tile([NP, 640], f32, tag="OS")
                for h in range(2):
                    wcols = NYX - h * 320 if h == 1 else 320  # 320, 305
                    tOC = pgo.tile([NP, 320], f32, tag="OC")
                    nc.tensor.matmul(tOC[:], lhs,
                                     t_g[:, h * 320:(h + 1) * 320],
                                     start=True, stop=True)
                    if h == 0:
                        nc.scalar.activation(tOS[:, 0:320], tOC[:], AF.Copy,
                                             scale=tRC[:, 0:1])
                    else:
                        nc.vector.tensor_scalar(tOS[:, 320:NYX],
                                                tOC[:, 0:wcols], tRC[:, 0:1],
                                                None, OP.mult)
                eng = nc.sync if zz % 2 == 0 else nc.scalar
                eng.dma_start(out=d_out[:, 12 + zz, :], in_=tOS[:, 0:NYX])
                if zz > 0:
                    eng2 = nc.scalar if zz % 2 == 0 else nc.sync
                    eng2.dma_start(out=d_out[:, 12 - zz, :],
                                   in_=tOS[:, 0:NYX])

    nc.finalize()
    _CACHE["nc"] = nc
    return nc


def _in_maps(params, consts):
    per = B // NCORES
    maps = []
    for i in range(NCORES):
        m = {"params": params[i * per:(i + 1) * per].reshape(NP, 2).copy()}
        m.update(consts)
        maps.append(m)
    return maps


def kernel(params):
    _ensure_paths()
    from concourse.bass_utils import run_bass_kernel_spmd

    params = np.asarray(params, dtype=np.float32)
    assert params.shape == (B, CH, 2)
    consts = _host_consts()
    nc = _build_nc()
    res = run_bass_kernel_spmd(nc, _in_maps(params, consts),
                               list(range(NCORES)))
    per = B // NCORES
    out = np.empty((B, CH, NZ, 25, 25), np.float32)
    for i in range(NCORES):
        out[i * per:(i + 1) * per] = res.results[i]["out"].reshape(
            per, CH, NZ, 25, 25)
    return out


def kernel_traced(params, tmpdir=None):
    """Run once with NTFF tracing; returns HW exec_time_ns (slowest core)."""
    _ensure_paths()
    from concourse.bass_utils import run_bass_kernel_spmd

    params = np.asarray(params, dtype=np.float32)
    consts = _host_consts()
    nc = _build_nc()
    res = run_bass_kernel_spmd(nc, _in_maps(params, consts),
                               list(range(NCORES)), trace=True, tmpdir=tmpdir)
    return res.exec_time_ns


# revision 18
# speedup vs baseline: 1.0409x; 1.0409x over previous
"""Born-Wolf PSF kernel for Trainium2, 8 NeuronCores, data-parallel over batch.

Self-contained: hardcodes all geometry from the problem spec.
  input : params (16, 64, 2) float32
  output: psf    (16, 64, 25, 25, 25) float32

Per (b,c) pair: psf = |trapz_rho J0(k n r rho) exp(-i 0.5 k rho^2 z n^2) rho|^2,
bilinearly interpolated from 35 anchor radii onto a 25x25 grid, reflect-padded
in z, and normalized.

v2 strategy: all per-(rho, pair*anchor) field quantities are sums of outer
products (rank-k separable), so they are computed by TensorE matmuls directly
into PSUM instead of DRAM-row broadcast DMAs (which serialized ~375us on one
DMA engine in v1). f32r matmuls truncate inputs to ~12 mantissa bits, so the
large phase factors (up to ~145 turns) are split hi/lo: hi holds 10 explicit
bits (exact in f32r), lo carries the remainder; the rank-5 expansion
(ones, hi*hi, hi*lo, lo*hi, lo*lo) restores full fp32 accuracy at 1 cycle/col.
Trapezoid weights and 1/2pi factors are folded into the matmul lhs constants;
normalization is folded into the per-partition scale of the PSUM->SBUF copy
after the G-expansion matmul.
"""
import os
import numpy as np

# ---------------- problem geometry (hardcoded) ----------------
B, CH = 16, 64
NCORES = 8
NP = (B // NCORES) * CH          # 128 pairs per core
NA, NJ, NZH, NZ = 35, 101, 13, 25
F = NP * NA                      # 4480
FCH = NP * NZH                   # 1664
NYX = 625
GW = 640                         # zero-padded G columns (2 x 320 matmuls)
PI = float(np.pi)
C0 = -0.1562499995e-1
C1 = -0.1098628627e-2
C2 = 0.1430488765e-3
S_AMP = float(np.sqrt(0.636619772))
SC1 = 20.0                       # power-row scale split (clamp = SC1^2 = 400)
QS = [1.0, -0.25, 0.015624999996, -0.00043402777473, 6.7816828549e-06,
      -6.781657507e-08, 4.7091319698e-10, -2.3995591574e-12,
      9.2118377553e-15, -2.3695100804e-17]
MAGIC = 12582912.0               # 1.5 * 2**23: (t+M)-M == round-to-nearest(t)
TMASK = 4.0 / (2.0 * PI) + 0.125  # mask threshold in turn units
VC = 8193.0                      # Veltkamp split const (2^13+1): hi keeps 10 bits

_CACHE = {}


def _split10(x):
    """Split f32 values into (hi, lo); hi has <=10 explicit mantissa bits."""
    xf = np.ascontiguousarray(np.asarray(x, np.float32))
    hi = (xf.view(np.uint32) & np.uint32(0xFFFFE000)).view(np.float32).copy()
    lo = (xf - hi).astype(np.float32)
    return hi, lo


def _host_consts():
    if "consts" in _CACHE:
        return _CACHE["consts"]
    f32 = np.float32
    R = (np.linspace(0, 34, NA) / 2.0).astype(np.float64)          # anchor radii
    RHO = np.linspace(0.0, 1.0, NJ).astype(np.float64)
    yp = xp = 12.0
    Y, X = np.meshgrid(np.arange(25.0), np.arange(25.0), indexing="ij")
    rPix = np.sqrt((X - xp) ** 2 + (Y - yp) ** 2)
    IDX1 = np.floor(rPix * 2).astype(np.int32)
    IDX2 = IDX1 + 1
    DISR1 = ((rPix - R[IDX1]) * 2).astype(np.float32).astype(np.float64)
    DISR2 = 1.0 - DISR1

    G = np.zeros((NA, GW), np.float64)
    for yy in range(25):
        for xx in range(25):
            yx = yy * 25 + xx
            G[IDX2[yy, xx], yx] += DISR1[yy, xx]
            G[IDX1[yy, xx], yx] += DISR2[yy, xx]
    gcol = G[:, :NYX].sum(1)
    w13 = np.concatenate([[1.0], np.full(NZH - 1, 2.0)])
    gw13 = np.zeros((NA, 2 * NZH))
    gw13[:, 0::2] = gcol[:, None] * w13[None, :]

    wt = np.full(NJ, 0.01, np.float64)
    wt[0] *= 0.5
    wt[-1] *= 0.5
    rw = RHO * wt                                    # trapezoid weight * rho

    with np.errstate(divide="ignore"):
        rinv = 1.0 / RHO
    rinv[0] = 0.0

    # phase lhs: t0 = 0.125 + RHO*knr_t + ps1*rinv/(2pi)^2 + ps3*rinv^3/(2pi)^4
    rho_hi, rho_lo = _split10(RHO)
    L_ph1 = np.stack([np.full(NJ, 0.125), rho_hi, rho_hi, rho_lo, rho_lo])
    L_ph2 = np.stack([rinv / (2 * np.pi) ** 2, rinv ** 3 / (2 * np.pi) ** 4])

    # poly lhs (J0 small branch, trapezoid weight folded in); row 0 <-> ones
    qrho = np.stack([QS[m] * (SC1 * SC1 * RHO ** 2) ** m for m in range(10)])
    L_poly = qrho * rw[None, :]

    # amplitude lhs (J0 large branch, trapezoid weight folded in)
    L_amp = np.stack([np.sqrt(rinv) * rw, -rinv ** 2.5 * rw])

    # C-matrix lhs: turns = RHO^2 * wcz_t  (+0.25 for cos; negated for sin)
    r2hi, r2lo = _split10(RHO ** 2)
    L_cos = np.stack([np.full(NJ, 0.25), r2hi, r2hi, r2lo, r2lo])
    L_sin = np.stack([np.zeros(NJ), -r2hi, -r2hi, -r2lo, -r2lo])

    Rinv = 1.0 / np.maximum(R, 1e-9)
    Rinv[0] = 0.0
    rtab = np.tile(R[None, :], (NP, 1))
    ri8c0 = np.tile((8.0 * C0 * Rinv)[None, :], (NP, 1))
    ri3c2 = np.tile((512.0 * C2 * Rinv ** 3)[None, :], (NP, 1))
    ztab = np.tile(np.arange(NZH, dtype=np.float64)[None, :], (NP, 1))

    consts = {
        "Lph1": L_ph1.astype(f32),
        "Lph2": L_ph2.astype(f32),
        "Lpoly": L_poly.astype(f32),
        "Lamp": L_amp.astype(f32),
        "Lcos": L_cos.astype(f32),
        "Lsin": L_sin.astype(f32),
        "rtab": rtab.astype(f32),
        "ri8c0": ri8c0.astype(f32),
        "ri3c2": ri3c2.astype(f32),
        "ztab": ztab.astype(f32),
        "gpad": G.astype(f32),
        "gw13": gw13.astype(f32),
    }
    for k, v in consts.items():
        assert np.isfinite(v).all(), k
    _CACHE["consts"] = consts
    return consts


def _ensure_paths():
    import sys
    for p in ("/opt/trn_rl_repo", "/root/.axon_site/_ro/trn_rl_repo"):
        if os.path.isdir(p) and p not in sys.path:
            sys.path.append(p)


def _build_nc():
    if "nc" in _CACHE:
        return _CACHE["nc"]
    _ensure_paths()
    from contextlib import ExitStack
    import concourse.bass as bass
    import concourse.bacc as bacc
    import concourse.tile as tile
    from concourse import mybir

    f32 = mybir.dt.float32
    f32r = mybir.dt.float32r
    bf16 = mybir.dt.bfloat16
    u8 = mybir.dt.uint8
    AF = mybir.ActivationFunctionType
    OP = mybir.AluOpType

    nc = bacc.Bacc()
    BIAS_A1 = float(np.log(S_AMP) - 0.5 * np.log(2 * np.pi))
    BIAS_A2 = float(np.log(64.0 * abs(C1) * S_AMP) - 2.5 * np.log(2 * np.pi))
    for val in (BIAS_A1, BIAS_A2):
        t = nc.alloc_sbuf_tensor(f"const-f32-{val}", [128, 1], f32)
        nc.gpsimd.memset(t.ap(), val)
        nc.const_aps.aps[(f32, val)] = t.ap()
    nc.all_engine_barrier()

    d_par = nc.declare_dram_parameter("params", [NP, 2], f32, isOutput=False)
    d_lph1 = nc.declare_dram_parameter("Lph1", [5, NJ], f32, isOutput=False)
    d_lph2 = nc.declare_dram_parameter("Lph2", [2, NJ], f32, isOutput=False)
    d_lpoly = nc.declare_dram_parameter("Lpoly", [10, NJ], f32, isOutput=False)
    d_lamp = nc.declare_dram_parameter("Lamp", [2, NJ], f32, isOutput=False)
    d_lcos = nc.declare_dram_parameter("Lcos", [5, NJ], f32, isOutput=False)
    d_lsin = nc.declare_dram_parameter("Lsin", [5, NJ], f32, isOutput=False)
    d_rtab = nc.declare_dram_parameter("rtab", [NP, NA], f32, isOutput=False)
    d_ri1 = nc.declare_dram_parameter("ri8c0", [NP, NA], f32, isOutput=False)
    d_ri3 = nc.declare_dram_parameter("ri3c2", [NP, NA], f32, isOutput=False)
    d_z = nc.declare_dram_parameter("ztab", [NP, NZH], f32, isOutput=False)
    d_g = nc.declare_dram_parameter("gpad", [NA, GW], f32, isOutput=False)
    d_gw = nc.declare_dram_parameter("gw13", [NA, 2 * NZH], f32,
                                     isOutput=False)
    d_out = nc.declare_dram_parameter("out", [NP, NZ, NYX], f32, isOutput=True)

    with tile.TileContext(nc) as tc, ExitStack() as ctx:
        p1 = ctx.enter_context(tc.tile_pool(name="p1", bufs=1))
        p2 = ctx.enter_context(tc.tile_pool(name="p2", bufs=3))

        # ---- const loads ----
        t_par = p1.tile([NP, 2], f32, tag="par")
        t_lph1f = p1.tile([5, NJ], f32, tag="lph1f")
        t_lph2f = p1.tile([2, NJ], f32, tag="lph2f")
        t_lpolyf = p1.tile([10, NJ], f32, tag="lpolyf")
        t_lampf = p1.tile([2, NJ], f32, tag="lampf")
        t_lcosf = p1.tile([5, NJ], f32, tag="lcosf")
        t_lsinf = p1.tile([5, NJ], f32, tag="lsinf")
        t_rtab = p1.tile([NP, NA], f32, tag="rtab")
        t_ri1 = p1.tile([NP, NA], f32, tag="ri1")
        t_ri3 = p1.tile([NP, NA], f32, tag="ri3")
        t_z = p1.tile([NP, NZH], f32, tag="ztab")
        t_gf = p1.tile([NA, GW], f32, tag="gpadf")
        t_gwf = p1.tile([NA, 2 * NZH], f32, tag="gw13f")
        for t, d in ((t_par, d_par), (t_lph1f, d_lph1), (t_lph2f, d_lph2),
                     (t_lpolyf, d_lpoly), (t_lampf, d_lamp), (t_lcosf, d_lcos),
                     (t_lsinf, d_lsin), (t_rtab, d_rtab), (t_ri1, d_ri1),
                     (t_ri3, d_ri3), (t_z, d_z), (t_gf, d_g), (t_gwf, d_gw)):
            nc.sync.dma_start(out=t[:], in_=d[:])
        t_lph1 = p1.tile([5, NJ], f32r, tag="lph1")
        t_lph2 = p1.tile([2, NJ], f32r, tag="lph2")
        t_lpoly = p1.tile([10, NJ], f32r, tag="lpoly")
        t_lamp = p1.tile([2, NJ], f32r, tag="lamp")
        t_lcos = p1.tile([5, NJ], f32r, tag="lcos")
        t_lsin = p1.tile([5, NJ], f32r, tag="lsin")
        t_g = p1.tile([NA, GW], f32r, tag="gpad")
        t_gw = p1.tile([NA, 2 * NZH], f32r, tag="gw13")
        for dst, srcf in ((t_lph1, t_lph1f), (t_lph2, t_lph2f),
                          (t_lpoly, t_lpolyf), (t_lamp, t_lampf),
                          (t_lcos, t_lcosf), (t_lsin, t_lsinf),
                          (t_g, t_gf), (t_gw, t_gwf)):
            nc.vector.tensor_copy(dst[:], srcf[:])

        # ---- pair-scalar stage ([NP,1] / [NP,NA]) ----
        # wcz chain first: rowsC feeds the C-stage matmuls, emitted earliest
        t_abs = p1.tile([NP, 2], f32, tag="pabs")
        nc.vector.scalar_tensor_tensor(t_abs[:], t_par[:], -1.0, t_par[:],
                                       OP.mult, OP.max)
        lam = t_abs[:, 0:1]
        enn = t_abs[:, 1:2]
        t_rl = p1.tile([NP, 1], f32, tag="rl")
        nc.vector.reciprocal(t_rl[:], lam)
        t_knt = p1.tile([NP, 1], f32, tag="knt")       # n/lam (turns per R*rho)
        nc.vector.tensor_tensor(t_knt[:], enn, t_rl[:], OP.mult)
        t_wct = p1.tile([NP, 1], f32, tag="wct")       # 0.5*n^2/lam
        nc.vector.scalar_tensor_tensor(t_wct[:], enn, 0.5, t_knt[:],
                                       OP.mult, OP.mult)
        # wcz in turns (<=87), Veltkamp split, f32r outputs for the rhs rows
        t_wcz = p1.tile([NP, NZH], f32, tag="wcz")
        nc.vector.tensor_scalar(t_wcz[:], t_z[:], t_wct[:], None, OP.mult)
        t_wv = p1.tile([NP, NZH], f32, tag="wv")
        nc.vector.tensor_scalar(t_wv[:], t_wcz[:], VC, None, OP.mult)
        t_wz2 = p1.tile([NP, NZH], f32, tag="wz2")
        nc.vector.tensor_tensor(t_wz2[:], t_wv[:], t_wcz[:], OP.subtract)
        t_whi = p1.tile([NP, NZH], f32, tag="whi")
        nc.vector.tensor_tensor(t_whi[:], t_wv[:], t_wz2[:], OP.subtract)
        t_whir = p1.tile([NP, NZH], f32r, tag="whir")
        nc.vector.tensor_tensor(t_whir[:], t_wv[:], t_wz2[:], OP.subtract)
        t_wlor = p1.tile([NP, NZH], f32r, tag="wlor")
        nc.vector.tensor_tensor(t_wlor[:], t_wcz[:], t_whi[:], OP.subtract)
        t_1f = p1.tile([NP, NA], f32, tag="onesf")
        nc.vector.memset(t_1f[:], 1.0)
        t_1r = p1.tile([NP, NA], f32r, tag="onesr")
        nc.vector.tensor_copy(t_1r[:], t_1f[:])
        rowsC = p1.tile([5, FCH], f32r, tag="rowsC")
        nc.scalar.dma_start(out=rowsC[0:1, :], in_=t_1r[:, 0:NZH])
        nc.scalar.dma_start(out=rowsC[1:2, :], in_=t_whir[:])
        nc.scalar.dma_start(out=rowsC[2:3, :], in_=t_wlor[:])
        nc.scalar.dma_start(out=rowsC[3:4, :], in_=rowsC[1:2, :])
        nc.scalar.dma_start(out=rowsC[4:5, :], in_=rowsC[2:3, :])

        # knr chain: phase rows (rowsH/rowsS) next, then amp/poly rows
        t_rkn = p1.tile([NP, 1], f32, tag="rkn")       # lam/n
        nc.vector.reciprocal(t_rkn[:], t_knt[:])
        t_knr = p1.tile([NP, NA], f32, tag="knr")      # knr in turns, <=145
        nc.vector.tensor_scalar(t_knr[:], t_rtab[:], t_knt[:], None, OP.mult)
        # Veltkamp split: hi keeps ~10 bits (exact under f32r rounding)
        t_kv = p1.tile([NP, NA], f32, tag="kv")
        nc.vector.tensor_scalar(t_kv[:], t_knr[:], VC, None, OP.mult)
        t_kz = p1.tile([NP, NA], f32, tag="kz")
        nc.vector.tensor_tensor(t_kz[:], t_kv[:], t_knr[:], OP.subtract)
        t_khi = p1.tile([NP, NA], f32, tag="khi")
        nc.vector.tensor_tensor(t_khi[:], t_kv[:], t_kz[:], OP.subtract)
        t_khir = p1.tile([NP, NA], f32r, tag="khir")
        nc.vector.tensor_tensor(t_khir[:], t_kv[:], t_kz[:], OP.subtract)
        t_klor = p1.tile([NP, NA], f32r, tag="klor")
        nc.vector.tensor_tensor(t_klor[:], t_knr[:], t_khi[:], OP.subtract)
        rowsH = p1.tile([5, F], f32r, tag="rowsH")    # ones | khi klo khi klo
        nc.sync.dma_start(out=rowsH[0:1, :], in_=t_1r[:])
        nc.sync.dma_start(out=rowsH[1:2, :], in_=t_khir[:])
        nc.sync.dma_start(out=rowsH[2:3, :], in_=t_klor[:])
        nc.sync.dma_start(out=rowsH[3:4, :], in_=rowsH[1:2, :])
        nc.sync.dma_start(out=rowsH[4:5, :], in_=rowsH[2:3, :])

        t_rkn3 = p1.tile([NP, 1], f32, tag="rkn3")
        nc.vector.tensor_tensor(t_rkn3[:], t_rkn[:], t_rkn[:], OP.mult)
        nc.vector.tensor_tensor(t_rkn3[:], t_rkn3[:], t_rkn[:], OP.mult)
        t_ps1r = p1.tile([NP, NA], f32r, tag="ps1r")
        nc.vector.tensor_scalar(t_ps1r[:], t_ri1[:], t_rkn[:], None, OP.mult)
        t_ps3r = p1.tile([NP, NA], f32r, tag="ps3r")
        nc.vector.tensor_scalar(t_ps3r[:], t_ri3[:], t_rkn3[:], None, OP.mult)
        rowsS = p1.tile([2, F], f32r, tag="rowsS")    # ps1 | ps3
        nc.sync.dma_start(out=rowsS[0:1, :], in_=t_ps1r[:])
        nc.sync.dma_start(out=rowsS[1:2, :], in_=t_ps3r[:])

        t_knm = p1.tile([NP, NA], f32, tag="knm")
        nc.vector.tensor_scalar_max(t_knm[:], t_knr[:], 1e-4)
        t_lk = p1.tile([NP, NA], f32, tag="lk")
        nc.scalar.activation(t_lk[:], t_knm[:], AF.Ln)
        t_a1 = p1.tile([NP, NA], f32, tag="a1")
        nc.scalar.activation(t_a1[:], t_lk[:], AF.Exp, bias=BIAS_A1, scale=-0.5)
        t_a2 = p1.tile([NP, NA], f32, tag="a2")
        nc.scalar.activation(t_a2[:], t_lk[:], AF.Exp, bias=BIAS_A2, scale=-2.5)
        t_a1r = p1.tile([NP, NA], f32r, tag="a1r")
        nc.vector.tensor_copy(t_a1r[:], t_a1[:])
        t_a2r = p1.tile([NP, NA], f32r, tag="a2r")
        nc.vector.tensor_copy(t_a2r[:], t_a2[:])
        rowsA = p1.tile([2, F], f32r, tag="rowsA")    # a1 | a2
        nc.sync.dma_start(out=rowsA[0:1, :], in_=t_a1r[:])
        nc.sync.dma_start(out=rowsA[1:2, :], in_=t_a2r[:])

        # power rows: v = min(knr_rad/SC1, SC1)^2 ; U[:, m*NA:(m+1)*NA] = v^(m+1)
        t_v0 = p1.tile([NP, NA], f32, tag="v0")
        nc.vector.tensor_scalar(t_v0[:], t_knr[:], 2.0 * PI / SC1, SC1,
                                OP.mult, OP.min)
        t_U = p1.tile([NP, 9 * NA], f32, tag="U")
        nc.vector.tensor_tensor(t_U[:, 0:NA], t_v0[:], t_v0[:], OP.mult)
        for m in range(1, 9):
            nc.vector.tensor_tensor(t_U[:, m * NA:(m + 1) * NA],
                                    t_U[:, (m - 1) * NA:m * NA],
                                    t_U[:, 0:NA], OP.mult)
        t_Ur = p1.tile([NP, 9 * NA], f32r, tag="Ur")
        nc.vector.tensor_copy(t_Ur[:], t_U[:])
        rowsP = p1.tile([10, F], f32r, tag="rowsP")   # ones | v^1..v^9
        nc.scalar.dma_start(out=rowsP[0:1, :], in_=t_1r[:])
        for m in range(9):
            nc.scalar.dma_start(out=rowsP[m + 1:m + 2, :],
                                in_=t_Ur[:, m * NA:(m + 1) * NA])

        # ---- field stage: per-chunk matmuls into PSUM + pointwise ----
        tJ0 = p1.tile([NJ, F], bf16, tag="J0")
        tMask = p1.tile([NJ, F], u8, tag="mask")
        tCT = p1.tile([NJ, NP * 26], bf16, tag="CT")
        ct3 = tCT[:].rearrange("p (n c) -> p n c", c=26)

        CW = 1024
        with tc.tile_pool(name="pf", bufs=2, space="PSUM") as pf:
            # ---- C matrices: cos/sin(2pi * rho^2 * wcz) -> CT bf16 ----
            nb = [0, 78, NP]
            for ci in range(2):
                n0, n1 = nb[ci], nb[ci + 1]
                w = (n1 - n0) * NZH
                slc = slice(n0 * NZH, n0 * NZH + w)
                chws = [(0, min(512, w))] + ([(512, w - 512)] if w > 512
                                              else [])
                for lhs, zoff, ptag in ((t_lcos, 0, "T0"), (t_lsin, NZH, "X")):
                    psCC = pf.tile([NJ, CW], f32, tag=ptag)
                    for o, hw in chws:
                        nc.tensor.matmul(
                            psCC[:, o:o + hw], lhs[:],
                            rowsC[:, n0 * NZH + o:n0 * NZH + o + hw],
                            start=True, stop=True)
                    tCRm = p2.tile([NJ, CW], f32, tag="RRm")
                    nc.scalar.activation(tCRm[:, 0:w], psCC[:, 0:w], AF.Copy,
                                         bias=MAGIC)
                    tCRR = p2.tile([NJ, CW], f32, tag="RR")
                    nc.scalar.activation(tCRR[:, 0:w], tCRm[:, 0:w], AF.Copy,
                                         bias=-MAGIC)
                    tCNU = p2.tile([NJ, CW], f32, tag="NU")
                    nc.vector.tensor_tensor(tCNU[:, 0:w], psCC[:, 0:w],
                                            tCRR[:, 0:w], OP.subtract)
                    nc.scalar.activation(
                        ct3[:, n0:n1, zoff:zoff + NZH],
                        tCNU[:, 0:w].rearrange("p (n z) -> p n z", z=NZH),
                        AF.Sin, scale=2.0 * PI)

            nchunks = (F + CW - 1) // CW
            for c in range(nchunks):
                w = min(CW, F - c * CW)
                sl = slice(c * CW, c * CW + w)
                hws = [(0, min(512, w))] + ([(512, w - 512)] if w > 512
                                             else [])
                psT0 = pf.tile([NJ, CW], f32, tag="T0")
                for o, hw in hws:
                    nc.tensor.matmul(psT0[:, o:o + hw], t_lph1[:],
                                     rowsH[:, c * CW + o:c * CW + o + hw],
                                     start=True, stop=False)
                # mask from the pure x-part (before asymptotic corrections)
                nc.vector.tensor_scalar(tMask[:, sl], psT0[:, 0:w], TMASK,
                                        None, OP.is_le)
                for o, hw in hws:
                    nc.tensor.matmul(psT0[:, o:o + hw], t_lph2[:],
                                     rowsS[:, c * CW + o:c * CW + o + hw],
                                     start=False, stop=True)
                tRRm = p2.tile([NJ, CW], f32, tag="RRm")
                nc.scalar.activation(tRRm[:, 0:w], psT0[:, 0:w], AF.Copy,
                                     bias=MAGIC)
                tRR = p2.tile([NJ, CW], f32, tag="RR")
                nc.scalar.activation(tRR[:, 0:w], tRRm[:, 0:w], AF.Copy,
                                     bias=-MAGIC)
                tNU = p2.tile([NJ, CW], f32, tag="NU")
                nc.vector.tensor_tensor(tNU[:, 0:w], psT0[:, 0:w],
                                        tRR[:, 0:w], OP.subtract)
                tCOS = p2.tile([NJ, CW], f32, tag="COS")
                nc.scalar.activation(tCOS[:, 0:w], tNU[:, 0:w], AF.Sin,
                                     scale=2.0 * PI)
                psX = pf.tile([NJ, CW], f32, tag="X")
                for o, hw in hws:
                    nc.tensor.matmul(psX[:, o:o + hw], t_lamp[:],
                                     rowsA[:, c * CW + o:c * CW + o + hw],
                                     start=True, stop=True)
                nc.vector.tensor_tensor(tJ0[:, sl], psX[:, 0:w],
                                        tCOS[:, 0:w], OP.mult)
                for o, hw in hws:
                    nc.tensor.matmul(psX[:, o:o + hw], t_lpoly[:],
                                     rowsP[:, c * CW + o:c * CW + o + hw],
                                     start=True, stop=True)
                nc.vector.copy_predicated(tJ0[:, sl], tMask[:, sl],
                                          psX[:, 0:w])

        # ---- per-pair contraction into PSUM, 4 waves of 32 pairs ----
        tUS = p1.tile([NA, FCH], f32, tag="plU")
        tVS = p1.tile([NA, FCH], f32, tag="plV")
        tPL = p1.tile([NA, FCH], f32r, tag="plP")
        us_z = tUS[:].rearrange("q (zz pp) -> q pp zz", pp=NP)
        vs_z = tVS[:].rearrange("q (zz pp) -> q pp zz", pp=NP)
        WP = 32
        with tc.tile_pool(name="ppr", bufs=2, space="PSUM") as ppr, \
                tc.tile_pool(name="pso", bufs=1, space="PSUM") as pso, \
                tc.tile_pool(name="pgo", bufs=2, space="PSUM") as pgo:
            for wv in range(NP // WP):
                tPRw = ppr.tile([NA, WP * 32], f32, tag="PR")
                for j in range(WP):
                    p = wv * WP + j
                    nc.tensor.matmul(tPRw[:, j * 32:j * 32 + 26],
                                     tJ0[:, p * NA:(p + 1) * NA],
                                     tCT[:, p * 26:(p + 1) * 26],
                                     start=True, stop=True)
                pr4 = tPRw[:].rearrange("q (n s) -> q n s", s=32)
                slw = slice(wv * WP, (wv + 1) * WP)
                nc.scalar.activation(us_z[:, slw, :], pr4[:, :, 0:NZH],
                                     AF.Square)
                nc.scalar.activation(vs_z[:, slw, :], pr4[:, :, NZH:26],
                                     AF.Square)
            nc.vector.tensor_tensor(tPL[:], tUS[:], tVS[:], OP.add)

            # ---- normalization: nrm[p] = sum_zz sum_a gw13[a,zz]*PL[a,zz*NP+p]
            psN = pso.tile([NP, 2], f32, tag="N")
            for zz in range(NZH):
                nc.tensor.matmul(psN[:], tPL[:, zz * NP:(zz + 1) * NP],
                                 t_gw[:, 2 * zz:2 * zz + 2],
                                 start=(zz == 0), stop=(zz == NZH - 1))
            tRC = p1.tile([NP, 1], f32, tag="RC")
            nc.vector.reciprocal(tRC[:], psN[:, 0:1])

            # ---- G expansion + normalize-on-copy + mirrored output ----
            for zz in range(NZH):
                lhs = tPL[:, zz * NP:(zz + 1) * NP]
                tOS = p2.tile([NP, 640], f32, tag="OS")
                for h in range(2):
                    wcols = NYX - h * 320 if h == 1 else 320  # 320, 305
                    tOC = pgo.tile([NP, 320], f32, tag="OC")
                    nc.tensor.matmul(tOC[:], lhs,
                                     t_g[:, h * 320:(h + 1) * 320],
                                     start=True, stop=True)
                    if h == 0:
                        nc.scalar.activation(tOS[:, 0:320], tOC[:], AF.Copy,
                                             scale=tRC[:, 0:1])
                    else:
                        nc.vector.tensor_scalar(tOS[:, 320:NYX],
                                                tOC[:, 0:wcols], tRC[:, 0:1],
                                                None, OP.mult)
                eng = nc.sync if zz % 2 == 0 else nc.scalar
                eng.dma_start(out=d_out[:, 12 + zz, :], in_=tOS[:, 0:NYX])
                if zz > 0:
                    eng2 = nc.scalar if zz % 2 == 0 else nc.sync
                    eng2.dma_start(out=d_out[:, 12 - zz, :],
                                   in_=tOS[:, 0:NYX])

    nc.finalize()
    _CACHE["nc"] = nc
    return nc


def _in_maps(params, consts):
    per = B // NCORES
    maps = []
    for i in range(NCORES):
        m = {"params": params[i * per:(i + 1) * per].reshape(NP, 2).copy()}
        m.update(consts)
        maps.append(m)
    return maps


def kernel(params):
    _ensure_paths()
    from concourse.bass_utils import run_bass_kernel_spmd

    params = np.asarray(params, dtype=np.float32)
    assert params.shape == (B, CH, 2)
    consts = _host_consts()
    nc = _build_nc()
    res = run_bass_kernel_spmd(nc, _in_maps(params, consts),
                               list(range(NCORES)))
    per = B // NCORES
    out = np.empty((B, CH, NZ, 25, 25), np.float32)
    for i in range(NCORES):
        out[i * per:(i + 1) * per] = res.results[i]["out"].reshape(
            per, CH, NZ, 25, 25)
    return out


def kernel_traced(params, tmpdir=None):
    """Run once with NTFF tracing; returns HW exec_time_ns (slowest core)."""
    _ensure_paths()
    from concourse.bass_utils import run_bass_kernel_spmd

    params = np.asarray(params, dtype=np.float32)
    consts = _host_consts()
    nc = _build_nc()
    res = run_bass_kernel_spmd(nc, _in_maps(params, consts),
                               list(range(NCORES)), trace=True, tmpdir=tmpdir)
    return res.exec_time_ns


# revision 19
# speedup vs baseline: 1.1231x; 1.0790x over previous
"""Born-Wolf PSF kernel for Trainium2, 8 NeuronCores, data-parallel over batch.

Self-contained: hardcodes all geometry from the problem spec.
  input : params (16, 64, 2) float32
  output: psf    (16, 64, 25, 25, 25) float32

Per (b,c) pair: psf = |trapz_rho J0(k n r rho) exp(-i 0.5 k rho^2 z n^2) rho|^2,
bilinearly interpolated from 35 anchor radii onto a 25x25 grid, reflect-padded
in z, and normalized.

v2 strategy: all per-(rho, pair*anchor) field quantities are sums of outer
products (rank-k separable), so they are computed by TensorE matmuls directly
into PSUM instead of DRAM-row broadcast DMAs (which serialized ~375us on one
DMA engine in v1). f32r matmuls truncate inputs to ~12 mantissa bits, so the
large phase factors (up to ~145 turns) are split hi/lo: hi holds 10 explicit
bits (exact in f32r), lo carries the remainder; the rank-5 expansion
(ones, hi*hi, hi*lo, lo*hi, lo*lo) restores full fp32 accuracy at 1 cycle/col.
Trapezoid weights and 1/2pi factors are folded into the matmul lhs constants;
normalization is folded into the per-partition scale of the PSUM->SBUF copy
after the G-expansion matmul.
"""
import os
import numpy as np

# ---------------- problem geometry (hardcoded) ----------------
B, CH = 16, 64
NCORES = 8
NP = (B // NCORES) * CH          # 128 pairs per core
NA, NJ, NZH, NZ = 35, 101, 13, 25
F = NP * NA                      # 4480
FCH = NP * NZH                   # 1664
NYX = 625
GW = 640                         # zero-padded G columns (2 x 320 matmuls)
PI = float(np.pi)
C0 = -0.1562499995e-1
C1 = -0.1098628627e-2
C2 = 0.1430488765e-3
S_AMP = float(np.sqrt(0.636619772))
SC1 = 20.0                       # power-row scale split (clamp = SC1^2 = 400)
QS = [1.0, -0.25, 0.015624999996, -0.00043402777473, 6.7816828549e-06,
      -6.781657507e-08, 4.7091319698e-10, -2.3995591574e-12,
      9.2118377553e-15, -2.3695100804e-17]
MAGIC = 12582912.0               # 1.5 * 2**23: (t+M)-M == round-to-nearest(t)
TMASK = 4.0 / (2.0 * PI) + 0.125  # mask threshold in turn units
VC = 8193.0                      # Veltkamp split const (2^13+1): hi keeps 10 bits

_CACHE = {}


def _split10(x):
    """Split f32 values into (hi, lo); hi has <=10 explicit mantissa bits."""
    xf = np.ascontiguousarray(np.asarray(x, np.float32))
    hi = (xf.view(np.uint32) & np.uint32(0xFFFFE000)).view(np.float32).copy()
    lo = (xf - hi).astype(np.float32)
    return hi, lo


def _host_consts():
    if "consts" in _CACHE:
        return _CACHE["consts"]
    f32 = np.float32
    R = (np.linspace(0, 34, NA) / 2.0).astype(np.float64)          # anchor radii
    RHO = np.linspace(0.0, 1.0, NJ).astype(np.float64)
    yp = xp = 12.0
    Y, X = np.meshgrid(np.arange(25.0), np.arange(25.0), indexing="ij")
    rPix = np.sqrt((X - xp) ** 2 + (Y - yp) ** 2)
    IDX1 = np.floor(rPix * 2).astype(np.int32)
    IDX2 = IDX1 + 1
    DISR1 = ((rPix - R[IDX1]) * 2).astype(np.float32).astype(np.float64)
    DISR2 = 1.0 - DISR1

    G = np.zeros((NA, GW), np.float64)
    for yy in range(25):
        for xx in range(25):
            yx = yy * 25 + xx
            G[IDX2[yy, xx], yx] += DISR1[yy, xx]
            G[IDX1[yy, xx], yx] += DISR2[yy, xx]
    gcol = G[:, :NYX].sum(1)
    w13 = np.concatenate([[1.0], np.full(NZH - 1, 2.0)])
    gw13 = np.zeros((NA, 2 * NZH))
    gw13[:, 0::2] = gcol[:, None] * w13[None, :]

    wt = np.full(NJ, 0.01, np.float64)
    wt[0] *= 0.5
    wt[-1] *= 0.5
    rw = RHO * wt                                    # trapezoid weight * rho

    with np.errstate(divide="ignore"):
        rinv = 1.0 / RHO
    rinv[0] = 0.0

    # phase lhs: t0 = 0.125 + RHO*knr_t + ps1*rinv/(2pi)^2 + ps3*rinv^3/(2pi)^4
    rho_hi, rho_lo = _split10(RHO)
    L_ph1 = np.stack([np.full(NJ, 0.125), rho_hi, rho_hi, rho_lo, rho_lo])
    L_ph2 = np.stack([rinv / (2 * np.pi) ** 2, rinv ** 3 / (2 * np.pi) ** 4])

    # poly lhs (J0 small branch, trapezoid weight folded in); row 0 <-> ones
    qrho = np.stack([QS[m] * (SC1 * SC1 * RHO ** 2) ** m for m in range(10)])
    L_poly = qrho * rw[None, :]

    # amplitude lhs (J0 large branch, trapezoid weight folded in)
    L_amp = np.stack([np.sqrt(rinv) * rw, -rinv ** 2.5 * rw])

    # C-matrix lhs: turns = RHO^2 * wcz_t  (+0.25 for cos; negated for sin)
    r2hi, r2lo = _split10(RHO ** 2)
    L_cos = np.stack([np.full(NJ, 0.25), r2hi, r2hi, r2lo, r2lo])
    L_sin = np.stack([np.zeros(NJ), -r2hi, -r2hi, -r2lo, -r2lo])

    Rinv = 1.0 / np.maximum(R, 1e-9)
    Rinv[0] = 0.0
    rtab = np.tile(R[None, :], (NP, 1))
    ri8c0 = np.tile((8.0 * C0 * Rinv)[None, :], (NP, 1))
    ri3c2 = np.tile((512.0 * C2 * Rinv ** 3)[None, :], (NP, 1))
    ztab = np.tile(np.arange(NZH, dtype=np.float64)[None, :], (NP, 1))

    # pack: tables128 [NP, 118] = rtab|ri8c0|ri3c2|ztab
    tab128 = np.concatenate([rtab, ri8c0, ri3c2, ztab], axis=1)
    # Lpack [10, 606] = lph1|lph2|lpoly|lamp|lcos|lsin (column blocks)
    lpack = np.zeros((10, 6 * NJ))
    for i, Lm in enumerate((L_ph1, L_ph2, L_poly, L_amp, L_cos, L_sin)):
        lpack[0:Lm.shape[0], i * NJ:(i + 1) * NJ] = Lm
    # Gpack [NA, 666] = gpad|gw13
    gpack = np.concatenate([G, gw13], axis=1)
    consts = {
        "tab128": tab128.astype(f32),
        "lpack": lpack.astype(f32),
        "gpack": gpack.astype(f32),
    }
    for k, v in consts.items():
        assert np.isfinite(v).all(), k
    _CACHE["consts"] = consts
    return consts


def _ensure_paths():
    import sys
    for p in ("/opt/trn_rl_repo", "/root/.axon_site/_ro/trn_rl_repo"):
        if os.path.isdir(p) and p not in sys.path:
            sys.path.append(p)


def _build_nc():
    if "nc" in _CACHE:
        return _CACHE["nc"]
    _ensure_paths()
    from contextlib import ExitStack
    import concourse.bass as bass
    import concourse.bacc as bacc
    import concourse.tile as tile
    from concourse import mybir

    f32 = mybir.dt.float32
    f32r = mybir.dt.float32r
    bf16 = mybir.dt.bfloat16
    u8 = mybir.dt.uint8
    AF = mybir.ActivationFunctionType
    OP = mybir.AluOpType

    nc = bacc.Bacc()
    BIAS_A1 = float(np.log(S_AMP) - 0.5 * np.log(2 * np.pi))
    BIAS_A2 = float(np.log(64.0 * abs(C1) * S_AMP) - 2.5 * np.log(2 * np.pi))
    for val in (BIAS_A1, BIAS_A2):
        t = nc.alloc_sbuf_tensor(f"const-f32-{val}", [128, 1], f32)
        nc.gpsimd.memset(t.ap(), val)
        nc.const_aps.aps[(f32, val)] = t.ap()
    nc.all_engine_barrier()

    d_par = nc.declare_dram_parameter("params", [NP, 2], f32, isOutput=False)
    d_tab = nc.declare_dram_parameter("tab128", [NP, 118], f32, isOutput=False)
    d_lp = nc.declare_dram_parameter("lpack", [10, 6 * NJ], f32,
                                     isOutput=False)
    d_gp = nc.declare_dram_parameter("gpack", [NA, GW + 2 * NZH], f32,
                                     isOutput=False)
    d_out = nc.declare_dram_parameter("out", [NP, NZ, NYX], f32, isOutput=True)

    with tile.TileContext(nc) as tc, ExitStack() as ctx:
        p1 = ctx.enter_context(tc.tile_pool(name="p1", bufs=1))
        p2 = ctx.enter_context(tc.tile_pool(name="p2", bufs=3))

        # ---- const loads (3 packed DMAs + params) ----
        t_par = p1.tile([NP, 2], f32, tag="par")
        t_tab = p1.tile([NP, 118], f32, tag="tab")
        t_lpf = p1.tile([10, 6 * NJ], f32, tag="lpf")
        t_gpf = p1.tile([NA, GW + 2 * NZH], f32, tag="gpf")
        nc.sync.dma_start(out=t_par[:], in_=d_par[:])
        nc.sync.dma_start(out=t_tab[:], in_=d_tab[:])
        nc.scalar.dma_start(out=t_lpf[:], in_=d_lp[:])
        nc.scalar.dma_start(out=t_gpf[:], in_=d_gp[:])
        t_lpr = p1.tile([10, 6 * NJ], f32r, tag="lpr")
        nc.vector.tensor_copy(t_lpr[:], t_lpf[:])
        t_gpr = p1.tile([NA, GW + 2 * NZH], f32r, tag="gpr")
        nc.vector.tensor_copy(t_gpr[:], t_gpf[:])
        t_rtab = t_tab[:, 0:NA]
        t_ri1 = t_tab[:, NA:2 * NA]
        t_ri3 = t_tab[:, 2 * NA:3 * NA]
        t_z = t_tab[:, 3 * NA:3 * NA + NZH]
        t_lph1 = t_lpr[0:5, 0:NJ]
        t_lph2 = t_lpr[0:2, NJ:2 * NJ]
        t_lpoly = t_lpr[0:10, 2 * NJ:3 * NJ]
        t_lamp = t_lpr[0:2, 3 * NJ:4 * NJ]
        t_lcos = t_lpr[0:5, 4 * NJ:5 * NJ]
        t_lsin = t_lpr[0:5, 5 * NJ:6 * NJ]
        t_g = t_gpr[:, 0:GW]
        t_gw = t_gpr[:, GW:GW + 2 * NZH]

        # ---- pair-scalar stage ([NP,1] / [NP,NA]) ----
        # wcz chain first: rowsC feeds the C-stage matmuls, emitted earliest
        t_abs = p1.tile([NP, 2], f32, tag="pabs")
        nc.vector.scalar_tensor_tensor(t_abs[:], t_par[:], -1.0, t_par[:],
                                       OP.mult, OP.max)
        lam = t_abs[:, 0:1]
        enn = t_abs[:, 1:2]
        t_rl = p1.tile([NP, 1], f32, tag="rl")
        nc.vector.reciprocal(t_rl[:], lam)
        t_knt = p1.tile([NP, 1], f32, tag="knt")       # n/lam (turns per R*rho)
        nc.vector.tensor_tensor(t_knt[:], enn, t_rl[:], OP.mult)
        t_wct = p1.tile([NP, 1], f32, tag="wct")       # 0.5*n^2/lam
        nc.vector.scalar_tensor_tensor(t_wct[:], enn, 0.5, t_knt[:],
                                       OP.mult, OP.mult)
        # wcz in turns (<=87), Veltkamp split, f32r outputs for the rhs rows
        t_wcz = p1.tile([NP, NZH], f32, tag="wcz")
        nc.vector.tensor_scalar(t_wcz[:], t_z, t_wct[:], None, OP.mult)
        t_wv = p1.tile([NP, NZH], f32, tag="wv")
        nc.vector.tensor_scalar(t_wv[:], t_wcz[:], VC, None, OP.mult)
        t_wz2 = p1.tile([NP, NZH], f32, tag="wz2")
        nc.vector.tensor_tensor(t_wz2[:], t_wv[:], t_wcz[:], OP.subtract)
        t_whi = p1.tile([NP, NZH], f32, tag="whi")
        nc.vector.tensor_tensor(t_whi[:], t_wv[:], t_wz2[:], OP.subtract)
        t_whir = p1.tile([NP, NZH], f32r, tag="whir")
        nc.vector.tensor_tensor(t_whir[:], t_wv[:], t_wz2[:], OP.subtract)
        t_wlor = p1.tile([NP, NZH], f32r, tag="wlor")
        nc.vector.tensor_tensor(t_wlor[:], t_wcz[:], t_whi[:], OP.subtract)
        t_1f = p1.tile([NP, NA], f32, tag="onesf")
        nc.vector.memset(t_1f[:], 1.0)
        t_1r = p1.tile([NP, NA], f32r, tag="onesr")
        nc.vector.tensor_copy(t_1r[:], t_1f[:])
        rowsC = p1.tile([5, FCH], f32r, tag="rowsC")
        nc.scalar.dma_start(out=rowsC[0:1, :], in_=t_1r[:, 0:NZH])
        nc.scalar.dma_start(out=rowsC[1:2, :], in_=t_whir[:])
        nc.scalar.dma_start(out=rowsC[2:3, :], in_=t_wlor[:])
        nc.scalar.dma_start(out=rowsC[3:4, :], in_=rowsC[1:2, :])
        nc.scalar.dma_start(out=rowsC[4:5, :], in_=rowsC[2:3, :])

        # knr chain: phase rows (rowsH/rowsS) next, then amp/poly rows
        t_rkn = p1.tile([NP, 1], f32, tag="rkn")       # lam/n
        nc.vector.reciprocal(t_rkn[:], t_knt[:])
        t_knr = p1.tile([NP, NA], f32, tag="knr")      # knr in turns, <=145
        nc.vector.tensor_scalar(t_knr[:], t_rtab, t_knt[:], None, OP.mult)
        # Veltkamp split: hi keeps ~10 bits (exact under f32r rounding)
        t_kv = p1.tile([NP, NA], f32, tag="kv")
        nc.vector.tensor_scalar(t_kv[:], t_knr[:], VC, None, OP.mult)
        t_kz = p1.tile([NP, NA], f32, tag="kz")
        nc.vector.tensor_tensor(t_kz[:], t_kv[:], t_knr[:], OP.subtract)
        t_khi = p1.tile([NP, NA], f32, tag="khi")
        nc.vector.tensor_tensor(t_khi[:], t_kv[:], t_kz[:], OP.subtract)
        t_khir = p1.tile([NP, NA], f32r, tag="khir")
        nc.vector.tensor_tensor(t_khir[:], t_kv[:], t_kz[:], OP.subtract)
        t_klor = p1.tile([NP, NA], f32r, tag="klor")
        nc.vector.tensor_tensor(t_klor[:], t_knr[:], t_khi[:], OP.subtract)
        rowsH = p1.tile([5, F], f32r, tag="rowsH")    # ones | khi klo khi klo
        nc.sync.dma_start(out=rowsH[0:1, :], in_=t_1r[:])
        nc.sync.dma_start(out=rowsH[1:2, :], in_=t_khir[:])
        nc.sync.dma_start(out=rowsH[2:3, :], in_=t_klor[:])
        nc.sync.dma_start(out=rowsH[3:4, :], in_=rowsH[1:2, :])
        nc.sync.dma_start(out=rowsH[4:5, :], in_=rowsH[2:3, :])

        t_rkn3 = p1.tile([NP, 1], f32, tag="rkn3")
        nc.vector.tensor_tensor(t_rkn3[:], t_rkn[:], t_rkn[:], OP.mult)
        nc.vector.tensor_tensor(t_rkn3[:], t_rkn3[:], t_rkn[:], OP.mult)
        t_ps1r = p1.tile([NP, NA], f32r, tag="ps1r")
        nc.vector.tensor_scalar(t_ps1r[:], t_ri1, t_rkn[:], None, OP.mult)
        t_ps3r = p1.tile([NP, NA], f32r, tag="ps3r")
        nc.vector.tensor_scalar(t_ps3r[:], t_ri3, t_rkn3[:], None, OP.mult)
        rowsS = p1.tile([2, F], f32r, tag="rowsS")    # ps1 | ps3
        nc.sync.dma_start(out=rowsS[0:1, :], in_=t_ps1r[:])
        nc.sync.dma_start(out=rowsS[1:2, :], in_=t_ps3r[:])

        t_knm = p1.tile([NP, NA], f32, tag="knm")
        nc.vector.tensor_scalar_max(t_knm[:], t_knr[:], 1e-4)
        t_lk = p1.tile([NP, NA], f32, tag="lk")
        nc.scalar.activation(t_lk[:], t_knm[:], AF.Ln)
        t_a1 = p1.tile([NP, NA], f32, tag="a1")
        nc.scalar.activation(t_a1[:], t_lk[:], AF.Exp, bias=BIAS_A1, scale=-0.5)
        t_a2 = p1.tile([NP, NA], f32, tag="a2")
        nc.scalar.activation(t_a2[:], t_lk[:], AF.Exp, bias=BIAS_A2, scale=-2.5)
        t_a1r = p1.tile([NP, NA], f32r, tag="a1r")
        nc.vector.tensor_copy(t_a1r[:], t_a1[:])
        t_a2r = p1.tile([NP, NA], f32r, tag="a2r")
        nc.vector.tensor_copy(t_a2r[:], t_a2[:])
        rowsA = p1.tile([2, F], f32r, tag="rowsA")    # a1 | a2
        nc.sync.dma_start(out=rowsA[0:1, :], in_=t_a1r[:])
        nc.sync.dma_start(out=rowsA[1:2, :], in_=t_a2r[:])

        # power rows: v = min(knr_rad/SC1, SC1)^2 ; U[:, m*NA:(m+1)*NA] = v^(m+1)
        t_v0 = p1.tile([NP, NA], f32, tag="v0")
        nc.vector.tensor_scalar(t_v0[:], t_knr[:], 2.0 * PI / SC1, SC1,
                                OP.mult, OP.min)
        t_U = p1.tile([NP, 9 * NA], f32, tag="U")
        nc.vector.tensor_tensor(t_U[:, 0:NA], t_v0[:], t_v0[:], OP.mult)
        for m in range(1, 9):
            nc.vector.tensor_tensor(t_U[:, m * NA:(m + 1) * NA],
                                    t_U[:, (m - 1) * NA:m * NA],
                                    t_U[:, 0:NA], OP.mult)
        t_Ur = p1.tile([NP, 9 * NA], f32r, tag="Ur")
        nc.vector.tensor_copy(t_Ur[:], t_U[:])
        rowsP = p1.tile([10, F], f32r, tag="rowsP")   # ones | v^1..v^9
        nc.scalar.dma_start(out=rowsP[0:1, :], in_=t_1r[:])
        for m in range(9):
            nc.scalar.dma_start(out=rowsP[m + 1:m + 2, :],
                                in_=t_Ur[:, m * NA:(m + 1) * NA])

        # ---- field stage: per-chunk matmuls into PSUM + pointwise ----
        tJ0 = p1.tile([NJ, F], bf16, tag="J0")
        tMask = p1.tile([NJ, F], u8, tag="mask")
        tCT = p1.tile([NJ, NP * 26], bf16, tag="CT")
        ct3 = tCT[:].rearrange("p (n c) -> p n c", c=26)

        CW = 1024
        with tc.tile_pool(name="pf", bufs=2, space="PSUM") as pf:
            # ---- C matrices: cos/sin(2pi * rho^2 * wcz) -> CT bf16 ----
            nb = [0, 78, NP]
            for ci in range(2):
                n0, n1 = nb[ci], nb[ci + 1]
                w = (n1 - n0) * NZH
                slc = slice(n0 * NZH, n0 * NZH + w)
                chws = [(0, min(512, w))] + ([(512, w - 512)] if w > 512
                                              else [])
                for lhs, zoff, ptag in ((t_lcos, 0, "T0"), (t_lsin, NZH, "X")):
                    psCC = pf.tile([NJ, CW], f32, tag=ptag)
                    for o, hw in chws:
                        nc.tensor.matmul(
                            psCC[:, o:o + hw], lhs,
                            rowsC[:, n0 * NZH + o:n0 * NZH + o + hw],
                            start=True, stop=True)
                    tCRR = p2.tile([NJ, CW], f32, tag="RR")
                    nc.vector.tensor_scalar(tCRR[:, 0:w], psCC[:, 0:w],
                                            MAGIC, MAGIC, OP.add, OP.subtract)
                    tCNU = p2.tile([NJ, CW], f32, tag="NU")
                    nc.vector.tensor_tensor(tCNU[:, 0:w], psCC[:, 0:w],
                                            tCRR[:, 0:w], OP.subtract)
                    nc.scalar.activation(
                        ct3[:, n0:n1, zoff:zoff + NZH],
                        tCNU[:, 0:w].rearrange("p (n z) -> p n z", z=NZH),
                        AF.Sin, scale=2.0 * PI)

            nchunks = (F + CW - 1) // CW
            for c in range(nchunks):
                w = min(CW, F - c * CW)
                sl = slice(c * CW, c * CW + w)
                hws = [(0, min(512, w))] + ([(512, w - 512)] if w > 512
                                             else [])
                psT0 = pf.tile([NJ, CW], f32, tag="T0")
                for o, hw in hws:
                    nc.tensor.matmul(psT0[:, o:o + hw], t_lph1,
                                     rowsH[:, c * CW + o:c * CW + o + hw],
                                     start=True, stop=False)
                # mask from the pure x-part (before asymptotic corrections)
                nc.vector.tensor_scalar(tMask[:, sl], psT0[:, 0:w], TMASK,
                                        None, OP.is_le)
                for o, hw in hws:
                    nc.tensor.matmul(psT0[:, o:o + hw], t_lph2,
                                     rowsS[:, c * CW + o:c * CW + o + hw],
                                     start=False, stop=True)
                tRRm = p2.tile([NJ, CW], f32, tag="RRm")
                nc.scalar.activation(tRRm[:, 0:w], psT0[:, 0:w], AF.Copy,
                                     bias=MAGIC)
                tRR = p2.tile([NJ, CW], f32, tag="RR")
                nc.scalar.activation(tRR[:, 0:w], tRRm[:, 0:w], AF.Copy,
                                     bias=-MAGIC)
                tNU = p2.tile([NJ, CW], f32, tag="NU")
                nc.vector.tensor_tensor(tNU[:, 0:w], psT0[:, 0:w],
                                        tRR[:, 0:w], OP.subtract)
                tCOS = p2.tile([NJ, CW], f32, tag="COS")
                nc.scalar.activation(tCOS[:, 0:w], tNU[:, 0:w], AF.Sin,
                                     scale=2.0 * PI)
                psX = pf.tile([NJ, CW], f32, tag="X")
                for o, hw in hws:
                    nc.tensor.matmul(psX[:, o:o + hw], t_lamp,
                                     rowsA[:, c * CW + o:c * CW + o + hw],
                                     start=True, stop=True)
                nc.vector.tensor_tensor(tJ0[:, sl], psX[:, 0:w],
                                        tCOS[:, 0:w], OP.mult)
                for o, hw in hws:
                    nc.tensor.matmul(psX[:, o:o + hw], t_lpoly,
                                     rowsP[:, c * CW + o:c * CW + o + hw],
                                     start=True, stop=True)
                nc.vector.copy_predicated(tJ0[:, sl], tMask[:, sl],
                                          psX[:, 0:w])

        # ---- per-pair contraction into PSUM, 4 waves of 32 pairs ----
        tUS = p1.tile([NA, FCH], f32, tag="plU")
        tVS = p1.tile([NA, FCH], f32, tag="plV")
        tPL = p1.tile([NA, FCH], f32r, tag="plP")
        us_z = tUS[:].rearrange("q (zz pp) -> q pp zz", pp=NP)
        vs_z = tVS[:].rearrange("q (zz pp) -> q pp zz", pp=NP)
        pl_z = tPL[:].rearrange("q (zz pp) -> q pp zz", pp=NP)
        WP = 32
        with tc.tile_pool(name="ppr", bufs=2, space="PSUM") as ppr, \
                tc.tile_pool(name="pso", bufs=1, space="PSUM") as pso, \
                tc.tile_pool(name="pgo", bufs=2, space="PSUM") as pgo:
            for wv in range(NP // WP):
                tPRw = ppr.tile([NA, WP * 32], f32, tag="PR")
                for j in range(WP):
                    p = wv * WP + j
                    nc.tensor.matmul(tPRw[:, j * 32:j * 32 + 26],
                                     tJ0[:, p * NA:(p + 1) * NA],
                                     tCT[:, p * 26:(p + 1) * 26],
                                     start=True, stop=True)
                pr4 = tPRw[:].rearrange("q (n s) -> q n s", s=32)
                slw = slice(wv * WP, (wv + 1) * WP)
                nc.scalar.activation(us_z[:, slw, :], pr4[:, :, 0:NZH],
                                     AF.Square)
                nc.scalar.activation(vs_z[:, slw, :], pr4[:, :, NZH:26],
                                     AF.Square)
                nc.vector.tensor_tensor(pl_z[:, slw, :], us_z[:, slw, :],
                                        vs_z[:, slw, :], OP.add)

            # ---- normalization: nrm[p] = sum_zz sum_a gw13[a,zz]*PL[a,zz*NP+p]
            psN = pso.tile([NP, 2], f32, tag="N")
            for zz in range(NZH):
                nc.tensor.matmul(psN[:], tPL[:, zz * NP:(zz + 1) * NP],
                                 t_gw[:, 2 * zz:2 * zz + 2],
                                 start=(zz == 0), stop=(zz == NZH - 1))
            tRC = p1.tile([NP, 1], f32, tag="RC")
            nc.vector.reciprocal(tRC[:], psN[:, 0:1])

            # ---- G expansion + normalize-on-copy + mirrored output ----
            for zz in range(NZH):
                lhs = tPL[:, zz * NP:(zz + 1) * NP]
                tOS = p2.tile([NP, 640], f32, tag="OS")
                for h in range(2):
                    wcols = NYX - h * 320 if h == 1 else 320  # 320, 305
                    tOC = pgo.tile([NP, 320], f32, tag="OC")
                    nc.tensor.matmul(tOC[:], lhs,
                                     t_g[:, h * 320:(h + 1) * 320],
                                     start=True, stop=True)
                    if h == 0:
                        nc.scalar.activation(tOS[:, 0:320], tOC[:], AF.Copy,
                                             scale=tRC[:, 0:1])
                    else:
                        nc.vector.tensor_scalar(tOS[:, 320:NYX],
                                                tOC[:, 0:wcols], tRC[:, 0:1],
                                                None, OP.mult)
                eng = nc.sync if zz % 2 == 0 else nc.scalar
                eng.dma_start(out=d_out[:, 12 + zz, :], in_=tOS[:, 0:NYX])
                if zz > 0:
                    eng2 = nc.scalar if zz % 2 == 0 else nc.sync
                    eng2.dma_start(out=d_out[:, 12 - zz, :],
                                   in_=tOS[:, 0:NYX])

    nc.finalize()
    _CACHE["nc"] = nc
    return nc


def _in_maps(params, consts):
    per = B // NCORES
    maps = []
    for i in range(NCORES):
        m = {"params": params[i * per:(i + 1) * per].reshape(NP, 2).copy()}
        m.update(consts)
        maps.append(m)
    return maps


def kernel(params):
    _ensure_paths()
    from concourse.bass_utils import run_bass_kernel_spmd

    params = np.asarray(params, dtype=np.float32)
    assert params.shape == (B, CH, 2)
    consts = _host_consts()
    nc = _build_nc()
    res = run_bass_kernel_spmd(nc, _in_maps(params, consts),
                               list(range(NCORES)))
    per = B // NCORES
    out = np.empty((B, CH, NZ, 25, 25), np.float32)
    for i in range(NCORES):
        out[i * per:(i + 1) * per] = res.results[i]["out"].reshape(
            per, CH, NZ, 25, 25)
    return out


def kernel_traced(params, tmpdir=None):
    """Run once with NTFF tracing; returns HW exec_time_ns (slowest core)."""
    _ensure_paths()
    from concourse.bass_utils import run_bass_kernel_spmd

    params = np.asarray(params, dtype=np.float32)
    consts = _host_consts()
    nc = _build_nc()
    res = run_bass_kernel_spmd(nc, _in_maps(params, consts),
                               list(range(NCORES)), trace=True, tmpdir=tmpdir)
    return res.exec_time_ns


# revision 23
# speedup vs baseline: 1.2803x; 1.1400x over previous
"""Born-Wolf PSF kernel for Trainium2, 8 NeuronCores, data-parallel over batch.

Self-contained: hardcodes all geometry from the problem spec.
  input : params (16, 64, 2) float32
  output: psf    (16, 64, 25, 25, 25) float32

Per (b,c) pair: psf = |trapz_rho J0(k n r rho) exp(-i 0.5 k rho^2 z n^2) rho|^2,
bilinearly interpolated from 35 anchor radii onto a 25x25 grid, reflect-padded
in z, and normalized.

v2 strategy: all per-(rho, pair*anchor) field quantities are sums of outer
products (rank-k separable), so they are computed by TensorE matmuls directly
into PSUM instead of DRAM-row broadcast DMAs (which serialized ~375us on one
DMA engine in v1). f32r matmuls truncate inputs to ~12 mantissa bits, so the
large phase factors (up to ~145 turns) are split hi/lo: hi holds 10 explicit
bits (exact in f32r), lo carries the remainder; the rank-5 expansion
(ones, hi*hi, hi*lo, lo*hi, lo*lo) restores full fp32 accuracy at 1 cycle/col.
Trapezoid weights and 1/2pi factors are folded into the matmul lhs constants;
normalization is folded into the per-partition scale of the PSUM->SBUF copy
after the G-expansion matmul.
"""
import os
import numpy as np

# ---------------- problem geometry (hardcoded) ----------------
B, CH = 16, 64
NCORES = 8
NP = (B // NCORES) * CH          # 128 pairs per core
NA, NJ, NZH, NZ = 35, 101, 13, 25
F = NP * NA                      # 4480
FCH = NP * NZH                   # 1664
NYX = 625
GW = 640                         # zero-padded G columns (2 x 320 matmuls)
PI = float(np.pi)
C0 = -0.1562499995e-1
C1 = -0.1098628627e-2
C2 = 0.1430488765e-3
S_AMP = float(np.sqrt(0.636619772))
SC1 = 20.0                       # power-row scale split (clamp = SC1^2 = 400)
QS = [1.0, -0.25, 0.015624999996, -0.00043402777473, 6.7816828549e-06,
      -6.781657507e-08, 4.7091319698e-10, -2.3995591574e-12,
      9.2118377553e-15, -2.3695100804e-17]
MAGIC = 12582912.0               # 1.5 * 2**23: (t+M)-M == round-to-nearest(t)
TMASK = 4.0 / (2.0 * PI) + 0.125  # mask threshold in turn units
VC = 8193.0                      # Veltkamp split const (2^13+1): hi keeps 10 bits
VC8 = 32769.0                    # Veltkamp (2^15+1): hi keeps 8 bits (bf16-exact)

_CACHE = {}


def _split8(x):
    """Split f32 values into (hi, lo); hi has <=8 explicit mantissa bits."""
    xf = np.ascontiguousarray(np.asarray(x, np.float32))
    hi = (xf.view(np.uint32) & np.uint32(0xFFFF8000)).view(np.float32).copy()
    lo = (xf - hi).astype(np.float32)
    return hi, lo


def _split8x3(x):
    """3-way split: x = a+b+c with a,b exactly bf16, c bf16-roundable."""
    a, r = _split8(x)
    b, c = _split8(r)
    return a, b, c


def _host_consts():
    if "consts" in _CACHE:
        return _CACHE["consts"]
    f32 = np.float32
    R = (np.linspace(0, 34, NA) / 2.0).astype(np.float64)          # anchor radii
    RHO = np.linspace(0.0, 1.0, NJ).astype(np.float64)
    yp = xp = 12.0
    Y, X = np.meshgrid(np.arange(25.0), np.arange(25.0), indexing="ij")
    rPix = np.sqrt((X - xp) ** 2 + (Y - yp) ** 2)
    IDX1 = np.floor(rPix * 2).astype(np.int32)
    IDX2 = IDX1 + 1
    DISR1 = ((rPix - R[IDX1]) * 2).astype(np.float32).astype(np.float64)
    DISR2 = 1.0 - DISR1

    G = np.zeros((NA, GW), np.float64)
    for yy in range(25):
        for xx in range(25):
            yx = yy * 25 + xx
            G[IDX2[yy, xx], yx] += DISR1[yy, xx]
            G[IDX1[yy, xx], yx] += DISR2[yy, xx]
    gcol = G[:, :NYX].sum(1)
    w13 = np.concatenate([[1.0], np.full(NZH - 1, 2.0)])
    gw13 = np.zeros((NA, 2 * NZH))
    gw13[:, 0::2] = gcol[:, None] * w13[None, :]

    wt = np.full(NJ, 0.01, np.float64)
    wt[0] *= 0.5
    wt[-1] *= 0.5
    rw = RHO * wt                                    # trapezoid weight * rho

    with np.errstate(divide="ignore"):
        rinv = 1.0 / RHO
    rinv[0] = 0.0

    # phase lhs: t0 = 0.125 + RHO*knr_t + ps1*rinv/(2pi)^2 + ps3*rinv^3/(2pi)^4
    # 3-way 8-bit splits (bf16-exact pieces) for both factors of RHO*knr_t
    r1, r2, r3 = _split8x3(RHO)
    L_ph1 = np.stack([np.full(NJ, 0.125), r1, r2, r3, r1, r2, r1])
    L_ph2 = np.stack([rinv / (2 * np.pi) ** 2, rinv ** 3 / (2 * np.pi) ** 4])

    # poly lhs (J0 small branch, trapezoid weight folded in); row 0 <-> ones
    qrho = np.stack([QS[m] * (SC1 * SC1 * RHO ** 2) ** m for m in range(10)])
    L_poly = qrho * rw[None, :]

    # amplitude lhs (J0 large branch, trapezoid weight folded in)
    L_amp = np.stack([np.sqrt(rinv) * rw, -rinv ** 2.5 * rw])

    # C-matrix lhs: turns = RHO^2 * wcz_t  (+0.25 for cos; negated for sin)
    q1, q2, q3 = _split8x3(RHO ** 2)
    L_cos = np.stack([np.full(NJ, 0.25), q1, q2, q3, q1, q2, q1])
    L_sin = np.stack([np.zeros(NJ), -q1, -q2, -q3, -q1, -q2, -q1])

    Rinv = 1.0 / np.maximum(R, 1e-9)
    Rinv[0] = 0.0
    rtab = np.tile(R[None, :], (NP, 1))
    ri8c0 = np.tile((8.0 * C0 * Rinv)[None, :], (NP, 1))
    ri3c2 = np.tile((512.0 * C2 * Rinv ** 3)[None, :], (NP, 1))
    ztab = np.tile(np.arange(NZH, dtype=np.float64)[None, :], (NP, 1))

    # pack: tables128 [NP, 118] = rtab|ri8c0|ri3c2|ztab
    tab128 = np.concatenate([rtab, ri8c0, ri3c2, ztab], axis=1)
    # Lb [7, 4*NJ] bf16 blocks = lph1|lph2(pad)|lcos|lsin; Lr = lpoly|lamp
    lb = np.zeros((7, 4 * NJ))
    for i, Lm in enumerate((L_ph1, L_ph2, L_cos, L_sin)):
        lb[0:Lm.shape[0], i * NJ:(i + 1) * NJ] = Lm
    lr = np.zeros((10, 2 * NJ))
    for i, Lm in enumerate((L_poly, L_amp)):
        lr[0:Lm.shape[0], i * NJ:(i + 1) * NJ] = Lm
    # Gpack [NA, 666] = gpad|gw13
    gpack = np.concatenate([G, gw13], axis=1)
    consts = {
        "tab128": tab128.astype(f32),
        "lbpack": lb.astype(f32),
        "lrpack": lr.astype(f32),
        "gpack": gpack.astype(f32),
    }
    for k, v in consts.items():
        assert np.isfinite(v).all(), k
    _CACHE["consts"] = consts
    return consts


def _ensure_paths():
    import sys
    for p in ("/opt/trn_rl_repo", "/root/.axon_site/_ro/trn_rl_repo"):
        if os.path.isdir(p) and p not in sys.path:
            sys.path.append(p)


def _build_nc():
    if "nc" in _CACHE:
        return _CACHE["nc"]
    _ensure_paths()
    from contextlib import ExitStack
    import concourse.bass as bass
    import concourse.bacc as bacc
    import concourse.tile as tile
    from concourse import mybir

    f32 = mybir.dt.float32
    f32r = mybir.dt.float32r
    bf16 = mybir.dt.bfloat16
    u8 = mybir.dt.uint8
    AF = mybir.ActivationFunctionType
    OP = mybir.AluOpType

    nc = bacc.Bacc()
    BIAS_A1 = float(np.log(S_AMP) - 0.5 * np.log(2 * np.pi))
    BIAS_A2 = float(np.log(64.0 * abs(C1) * S_AMP) - 2.5 * np.log(2 * np.pi))
    for val in (BIAS_A1, BIAS_A2):
        t = nc.alloc_sbuf_tensor(f"const-f32-{val}", [128, 1], f32)
        nc.gpsimd.memset(t.ap(), val)
        nc.const_aps.aps[(f32, val)] = t.ap()
    nc.all_engine_barrier()

    d_par = nc.declare_dram_parameter("params", [NP, 2], f32, isOutput=False)
    d_tab = nc.declare_dram_parameter("tab128", [NP, 118], f32, isOutput=False)
    d_lb = nc.declare_dram_parameter("lbpack", [7, 4 * NJ], f32,
                                     isOutput=False)
    d_lr = nc.declare_dram_parameter("lrpack", [10, 2 * NJ], f32,
                                     isOutput=False)
    d_gp = nc.declare_dram_parameter("gpack", [NA, GW + 2 * NZH], f32,
                                     isOutput=False)
    d_out = nc.declare_dram_parameter("out", [NP, NZ, NYX], f32, isOutput=True)

    with tile.TileContext(nc) as tc, ExitStack() as ctx:
        p1 = ctx.enter_context(tc.tile_pool(name="p1", bufs=1))
        p2 = ctx.enter_context(tc.tile_pool(name="p2", bufs=3))

        # ---- const loads (4 packed DMAs + params) ----
        t_par = p1.tile([NP, 2], f32, tag="par")
        t_tab = p1.tile([NP, 118], f32, tag="tab")
        t_lbf = p1.tile([7, 4 * NJ], f32, tag="lbf")
        t_lrf = p1.tile([10, 2 * NJ], f32, tag="lrf")
        t_gpf = p1.tile([NA, GW + 2 * NZH], f32, tag="gpf")
        nc.sync.dma_start(out=t_par[:], in_=d_par[:])
        nc.sync.dma_start(out=t_tab[:], in_=d_tab[:])
        nc.scalar.dma_start(out=t_lbf[:], in_=d_lb[:])
        nc.scalar.dma_start(out=t_lrf[:], in_=d_lr[:])
        nc.scalar.dma_start(out=t_gpf[:], in_=d_gp[:])
        t_rtab = t_tab[:, 0:NA]
        t_ri1 = t_tab[:, NA:2 * NA]
        t_ri3 = t_tab[:, 2 * NA:3 * NA]
        t_z = t_tab[:, 3 * NA:3 * NA + NZH]

        # ---- pair-scalar stage ([NP,1] / [NP,NA]) ----
        # wcz chain first: rowsC feeds the C-stage matmuls, emitted earliest
        t_abs = p1.tile([NP, 2], f32, tag="pabs")
        nc.vector.scalar_tensor_tensor(t_abs[:], t_par[:], -1.0, t_par[:],
                                       OP.mult, OP.max)
        lam = t_abs[:, 0:1]
        enn = t_abs[:, 1:2]
        t_rl = p1.tile([NP, 1], f32, tag="rl")
        nc.vector.reciprocal(t_rl[:], lam)
        t_knt = p1.tile([NP, 1], f32, tag="knt")       # n/lam (turns per R*rho)
        nc.vector.tensor_tensor(t_knt[:], enn, t_rl[:], OP.mult)
        t_wct = p1.tile([NP, 1], f32, tag="wct")       # 0.5*n^2/lam
        nc.vector.scalar_tensor_tensor(t_wct[:], enn, 0.5, t_knt[:],
                                       OP.mult, OP.mult)

        def velt3(x, n, cols, tag):
            """3-way 8-bit Veltkamp split -> three bf16 tiles (exact pieces)."""
            y = p1.tile([NP, cols], f32, tag=tag + "y")
            nc.vector.tensor_scalar(y[:], x, VC8, None, OP.mult)
            z = p1.tile([NP, cols], f32, tag=tag + "z")
            nc.vector.tensor_tensor(z[:], y[:], x, OP.subtract)
            h1 = p1.tile([NP, cols], f32, tag=tag + "h1")
            nc.vector.tensor_tensor(h1[:], y[:], z[:], OP.subtract)
            b1 = p1.tile([NP, cols], bf16, tag=tag + "b1")
            nc.vector.tensor_tensor(b1[:], y[:], z[:], OP.subtract)
            rem = p1.tile([NP, cols], f32, tag=tag + "rm")
            nc.vector.tensor_tensor(rem[:], x, h1[:], OP.subtract)
            y2 = p1.tile([NP, cols], f32, tag=tag + "y2")
            nc.vector.tensor_scalar(y2[:], rem[:], VC8, None, OP.mult)
            z2 = p1.tile([NP, cols], f32, tag=tag + "z2")
            nc.vector.tensor_tensor(z2[:], y2[:], rem[:], OP.subtract)
            h2 = p1.tile([NP, cols], f32, tag=tag + "h2")
            nc.vector.tensor_tensor(h2[:], y2[:], z2[:], OP.subtract)
            b2 = p1.tile([NP, cols], bf16, tag=tag + "b2")
            nc.vector.tensor_tensor(b2[:], y2[:], z2[:], OP.subtract)
            b3 = p1.tile([NP, cols], bf16, tag=tag + "b3")
            nc.vector.tensor_tensor(b3[:], rem[:], h2[:], OP.subtract)
            return b1, b2, b3

        # wcz in turns (<=87), 3-way split into bf16 rows
        t_wcz = p1.tile([NP, NZH], f32, tag="wcz")
        nc.vector.tensor_scalar(t_wcz[:], t_z, t_wct[:], None, OP.mult)
        w1, w2, w3 = velt3(t_wcz[:], NP, NZH, "w")
        t_1f = p1.tile([NP, NA], f32, tag="onesf")
        nc.vector.memset(t_1f[:], 1.0)
        t_1b = p1.tile([NP, NA], bf16, tag="onesb")
        nc.vector.tensor_copy(t_1b[:], t_1f[:])
        rowsC = p1.tile([7, FCH], bf16, tag="rowsC")  # ones w1 w1 w1 w2 w2 w3
        nc.scalar.dma_start(out=rowsC[0:1, :], in_=t_1b[:, 0:NZH])
        nc.scalar.dma_start(out=rowsC[1:2, :], in_=w1[:])
        nc.scalar.dma_start(out=rowsC[4:5, :], in_=w2[:])
        nc.scalar.dma_start(out=rowsC[6:7, :], in_=w3[:])
        nc.scalar.dma_start(out=rowsC[2:3, :], in_=rowsC[1:2, :])
        nc.scalar.dma_start(out=rowsC[3:4, :], in_=rowsC[1:2, :])
        nc.scalar.dma_start(out=rowsC[5:6, :], in_=rowsC[4:5, :])

        # L tables: bf16 pack conversion (phase + C), f32r pack (poly + amp)
        t_lbb = p1.tile([7, 4 * NJ], bf16, tag="lbb")
        nc.vector.tensor_copy(t_lbb[:], t_lbf[:])
        t_lph1 = t_lbb[0:7, 0:NJ]
        t_lph2b = t_lbb[0:2, NJ:2 * NJ]
        t_lcos = t_lbb[0:7, 2 * NJ:3 * NJ]
        t_lsin = t_lbb[0:7, 3 * NJ:4 * NJ]

        # knr chain: phase rows (rowsH/rowsS) next, then amp/poly rows
        t_rkn = p1.tile([NP, 1], f32, tag="rkn")       # lam/n
        nc.vector.reciprocal(t_rkn[:], t_knt[:])
        t_knr = p1.tile([NP, NA], f32, tag="knr")      # knr in turns, <=145
        nc.vector.tensor_scalar(t_knr[:], t_rtab, t_knt[:], None, OP.mult)
        k1, k2, k3 = velt3(t_knr[:], NP, NA, "k")
        rowsH = p1.tile([7, F], bf16, tag="rowsH")    # ones k1 k1 k1 k2 k2 k3
        nc.sync.dma_start(out=rowsH[0:1, :], in_=t_1b[:])
        nc.sync.dma_start(out=rowsH[1:2, :], in_=k1[:])
        nc.sync.dma_start(out=rowsH[4:5, :], in_=k2[:])
        nc.sync.dma_start(out=rowsH[6:7, :], in_=k3[:])
        nc.sync.dma_start(out=rowsH[2:3, :], in_=rowsH[1:2, :])
        nc.sync.dma_start(out=rowsH[3:4, :], in_=rowsH[1:2, :])
        nc.sync.dma_start(out=rowsH[5:6, :], in_=rowsH[4:5, :])

        t_rkn3 = p1.tile([NP, 1], f32, tag="rkn3")
        nc.vector.tensor_tensor(t_rkn3[:], t_rkn[:], t_rkn[:], OP.mult)
        nc.vector.tensor_tensor(t_rkn3[:], t_rkn3[:], t_rkn[:], OP.mult)
        t_ps1b = p1.tile([NP, NA], bf16, tag="ps1b")
        nc.vector.tensor_scalar(t_ps1b[:], t_ri1, t_rkn[:], None, OP.mult)
        t_ps3b = p1.tile([NP, NA], bf16, tag="ps3b")
        nc.vector.tensor_scalar(t_ps3b[:], t_ri3, t_rkn3[:], None, OP.mult)
        rowsS = p1.tile([2, F], bf16, tag="rowsS")    # ps1 | ps3
        nc.sync.dma_start(out=rowsS[0:1, :], in_=t_ps1b[:])
        nc.sync.dma_start(out=rowsS[1:2, :], in_=t_ps3b[:])

        # f32r pack conversion (poly + amp lhs)
        t_lrr = p1.tile([10, 2 * NJ], f32r, tag="lrr")
        nc.vector.tensor_copy(t_lrr[:], t_lrf[:])
        t_lpoly = t_lrr[0:10, 0:NJ]
        t_lamp = t_lrr[0:2, NJ:2 * NJ]

        t_knm = p1.tile([NP, NA], f32, tag="knm")
        nc.vector.tensor_scalar_max(t_knm[:], t_knr[:], 1e-4)
        t_lk = p1.tile([NP, NA], f32, tag="lk")
        nc.scalar.activation(t_lk[:], t_knm[:], AF.Ln)
        t_a1 = p1.tile([NP, NA], f32, tag="a1")
        nc.scalar.activation(t_a1[:], t_lk[:], AF.Exp, bias=BIAS_A1, scale=-0.5)
        t_a2 = p1.tile([NP, NA], f32, tag="a2")
        nc.scalar.activation(t_a2[:], t_lk[:], AF.Exp, bias=BIAS_A2, scale=-2.5)
        t_a1r = p1.tile([NP, NA], f32r, tag="a1r")
        nc.vector.tensor_copy(t_a1r[:], t_a1[:])
        t_a2r = p1.tile([NP, NA], f32r, tag="a2r")
        nc.vector.tensor_copy(t_a2r[:], t_a2[:])
        rowsA = p1.tile([2, F], f32r, tag="rowsA")    # a1 | a2
        nc.sync.dma_start(out=rowsA[0:1, :], in_=t_a1r[:])
        nc.sync.dma_start(out=rowsA[1:2, :], in_=t_a2r[:])

        # power rows: v = min(knr_rad/SC1, SC1)^2 ; U[:, m*NA:(m+1)*NA] = v^(m+1)
        t_v0 = p1.tile([NP, NA], f32, tag="v0")
        nc.vector.tensor_scalar(t_v0[:], t_knr[:], 2.0 * PI / SC1, SC1,
                                OP.mult, OP.min)
        t_U = p1.tile([NP, 9 * NA], f32, tag="U")
        nc.vector.tensor_tensor(t_U[:, 0:NA], t_v0[:], t_v0[:], OP.mult)
        for m in range(1, 9):
            nc.vector.tensor_tensor(t_U[:, m * NA:(m + 1) * NA],
                                    t_U[:, (m - 1) * NA:m * NA],
                                    t_U[:, 0:NA], OP.mult)
        t_Ur = p1.tile([NP, 9 * NA], f32r, tag="Ur")
        nc.vector.tensor_copy(t_Ur[:], t_U[:])
        t_1r = p1.tile([NP, NA], f32r, tag="onesr")
        nc.vector.tensor_copy(t_1r[:], t_1f[:])
        rowsP = p1.tile([10, F], f32r, tag="rowsP")   # ones | v^1..v^9
        nc.scalar.dma_start(out=rowsP[0:1, :], in_=t_1r[:])
        for m in range(9):
            nc.scalar.dma_start(out=rowsP[m + 1:m + 2, :],
                                in_=t_Ur[:, m * NA:(m + 1) * NA])

        # G tables in bf16 (G expansion + norm matmuls)
        t_gpb = p1.tile([NA, GW + 2 * NZH], bf16, tag="gpb")
        nc.vector.tensor_copy(t_gpb[:], t_gpf[:])
        t_g = t_gpb[:, 0:GW]
        t_gw = t_gpb[:, GW:GW + 2 * NZH]

        # ---- field stage: per-chunk matmuls into PSUM + pointwise ----
        tJ0 = p1.tile([NJ, F], bf16, tag="J0")
        tMask = p1.tile([NJ, F], u8, tag="mask")
        tCT = p1.tile([NJ, NP * 26], bf16, tag="CT")
        ct3 = tCT[:].rearrange("p (n c) -> p n c", c=26)

        CW = 1024
        with tc.tile_pool(name="pf", bufs=2, space="PSUM") as pf:
            # ---- C matrices: cos/sin(2pi * rho^2 * wcz) -> CT bf16 ----
            nb = [0, 78, NP]
            for ci in range(2):
                n0, n1 = nb[ci], nb[ci + 1]
                w = (n1 - n0) * NZH
                slc = slice(n0 * NZH, n0 * NZH + w)
                chws = [(0, min(512, w))] + ([(512, w - 512)] if w > 512
                                              else [])
                for lhs, zoff, ptag in ((t_lcos, 0, "T0"), (t_lsin, NZH, "X")):
                    psCC = pf.tile([NJ, CW], f32, tag=ptag)
                    for o, hw in chws:
                        nc.tensor.matmul(
                            psCC[:, o:o + hw], lhs,
                            rowsC[:, n0 * NZH + o:n0 * NZH + o + hw],
                            start=True, stop=True)
                    tCRR = p2.tile([NJ, CW], f32, tag="RR")
                    nc.vector.tensor_scalar(tCRR[:, 0:w], psCC[:, 0:w],
                                            MAGIC, MAGIC, OP.add, OP.subtract)
                    tCNU = p2.tile([NJ, CW], f32, tag="NU")
                    nc.vector.tensor_tensor(tCNU[:, 0:w], psCC[:, 0:w],
                                            tCRR[:, 0:w], OP.subtract)
                    nc.scalar.activation(
                        ct3[:, n0:n1, zoff:zoff + NZH],
                        tCNU[:, 0:w].rearrange("p (n z) -> p n z", z=NZH),
                        AF.Sin, scale=2.0 * PI)

            nchunks = (F + CW - 1) // CW
            for c in range(nchunks):
                w = min(CW, F - c * CW)
                sl = slice(c * CW, c * CW + w)
                hws = [(0, min(512, w))] + ([(512, w - 512)] if w > 512
                                             else [])
                psT0 = pf.tile([NJ, CW], f32, tag="T0")
                for o, hw in hws:
                    nc.tensor.matmul(psT0[:, o:o + hw], t_lph1,
                                     rowsH[:, c * CW + o:c * CW + o + hw],
                                     start=True, stop=False)
                # mask from the pure x-part (before asymptotic corrections)
                nc.vector.tensor_scalar(tMask[:, sl], psT0[:, 0:w], TMASK,
                                        None, OP.is_le)
                for o, hw in hws:
                    nc.tensor.matmul(psT0[:, o:o + hw], t_lph2b,
                                     rowsS[:, c * CW + o:c * CW + o + hw],
                                     start=False, stop=True)
                tRRm = p2.tile([NJ, CW], f32, tag="RRm")
                nc.scalar.activation(tRRm[:, 0:w], psT0[:, 0:w], AF.Copy,
                                     bias=MAGIC)
                tRR = p2.tile([NJ, CW], f32, tag="RR")
                nc.scalar.activation(tRR[:, 0:w], tRRm[:, 0:w], AF.Copy,
                                     bias=-MAGIC)
                tNU = p2.tile([NJ, CW], f32, tag="NU")
                nc.vector.tensor_tensor(tNU[:, 0:w], psT0[:, 0:w],
                                        tRR[:, 0:w], OP.subtract)
                tCOS = p2.tile([NJ, CW], f32, tag="COS")
                nc.scalar.activation(tCOS[:, 0:w], tNU[:, 0:w], AF.Sin,
                                     scale=2.0 * PI)
                psX = pf.tile([NJ, CW], f32, tag="X")
                for o, hw in hws:
                    nc.tensor.matmul(psX[:, o:o + hw], t_lamp,
                                     rowsA[:, c * CW + o:c * CW + o + hw],
                                     start=True, stop=True)
                nc.vector.tensor_tensor(tJ0[:, sl], psX[:, 0:w],
                                        tCOS[:, 0:w], OP.mult)
                for o, hw in hws:
                    nc.tensor.matmul(psX[:, o:o + hw], t_lpoly,
                                     rowsP[:, c * CW + o:c * CW + o + hw],
                                     start=True, stop=True)
                nc.vector.copy_predicated(tJ0[:, sl], tMask[:, sl],
                                          psX[:, 0:w])

        # ---- per-pair contraction into PSUM, 4 waves of 32 pairs ----
        tUS = p1.tile([NA, FCH], f32, tag="plU")
        tVS = p1.tile([NA, FCH], f32, tag="plV")
        tPL = p1.tile([NA, FCH], bf16, tag="plP")
        us_z = tUS[:].rearrange("q (zz pp) -> q pp zz", pp=NP)
        vs_z = tVS[:].rearrange("q (zz pp) -> q pp zz", pp=NP)
        pl_z = tPL[:].rearrange("q (zz pp) -> q pp zz", pp=NP)
        WP = 32
        with tc.tile_pool(name="ppr", bufs=2, space="PSUM") as ppr, \
                tc.tile_pool(name="pso", bufs=1, space="PSUM") as pso, \
                tc.tile_pool(name="pgo", bufs=2, space="PSUM") as pgo:
            for wv in range(NP // WP):
                tPRw = ppr.tile([NA, WP * 32], f32, tag="PR")
                for j in range(WP):
                    p = wv * WP + j
                    nc.tensor.matmul(tPRw[:, j * 32:j * 32 + 26],
                                     tJ0[:, p * NA:(p + 1) * NA],
                                     tCT[:, p * 26:(p + 1) * 26],
                                     start=True, stop=True)
                pr4 = tPRw[:].rearrange("q (n s) -> q n s", s=32)
                slw = slice(wv * WP, (wv + 1) * WP)
                nc.scalar.activation(us_z[:, slw, :], pr4[:, :, 0:NZH],
                                     AF.Square)
                nc.scalar.activation(vs_z[:, slw, :], pr4[:, :, NZH:26],
                                     AF.Square)
                nc.vector.tensor_tensor(pl_z[:, slw, :], us_z[:, slw, :],
                                        vs_z[:, slw, :], OP.add)

            # ---- normalization: nrm[p] = sum_zz sum_a gw13[a,zz]*PL[a,zz*NP+p]
            psN = pso.tile([NP, 2], f32, tag="N")
            for zz in range(NZH):
                nc.tensor.matmul(psN[:], tPL[:, zz * NP:(zz + 1) * NP],
                                 t_gw[:, 2 * zz:2 * zz + 2],
                                 start=(zz == 0), stop=(zz == NZH - 1))
            tRC = p1.tile([NP, 1], f32, tag="RC")
            nc.vector.reciprocal(tRC[:], psN[:, 0:1])

            # ---- G expansion + normalize-on-copy + mirrored output ----
            for zz in range(NZH):
                lhs = tPL[:, zz * NP:(zz + 1) * NP]
                tOS = p2.tile([NP, 640], f32, tag="OS")
                for h in range(2):
                    wcols = NYX - h * 320 if h == 1 else 320  # 320, 305
                    tOC = pgo.tile([NP, 320], f32, tag="OC")
                    nc.tensor.matmul(tOC[:], lhs,
                                     t_g[:, h * 320:(h + 1) * 320],
                                     start=True, stop=True)
                    if h == 0:
                        nc.scalar.activation(tOS[:, 0:320], tOC[:], AF.Copy,
                                             scale=tRC[:, 0:1])
                    else:
                        nc.vector.tensor_scalar(tOS[:, 320:NYX],
                                                tOC[:, 0:wcols], tRC[:, 0:1],
                                                None, OP.mult)
                eng = nc.sync if zz % 2 == 0 else nc.scalar
                eng.dma_start(out=d_out[:, 12 + zz, :], in_=tOS[:, 0:NYX])
                if zz > 0:
                    eng2 = nc.scalar if zz % 2 == 0 else nc.sync
                    eng2.dma_start(out=d_out[:, 12 - zz, :],
                                   in_=tOS[:, 0:NYX])

    nc.finalize()
    _CACHE["nc"] = nc
    return nc


def _in_maps(params, consts):
    per = B // NCORES
    maps = []
    for i in range(NCORES):
        m = {"params": params[i * per:(i + 1) * per].reshape(NP, 2).copy()}
        m.update(consts)
        maps.append(m)
    return maps


def kernel(params):
    _ensure_paths()
    from concourse.bass_utils import run_bass_kernel_spmd

    params = np.asarray(params, dtype=np.float32)
    assert params.shape == (B, CH, 2)
    consts = _host_consts()
    nc = _build_nc()
    res = run_bass_kernel_spmd(nc, _in_maps(params, consts),
                               list(range(NCORES)))
    per = B // NCORES
    out = np.empty((B, CH, NZ, 25, 25), np.float32)
    for i in range(NCORES):
        out[i * per:(i + 1) * per] = res.results[i]["out"].reshape(
            per, CH, NZ, 25, 25)
    return out


def kernel_traced(params, tmpdir=None):
    """Run once with NTFF tracing; returns HW exec_time_ns (slowest core)."""
    _ensure_paths()
    from concourse.bass_utils import run_bass_kernel_spmd

    params = np.asarray(params, dtype=np.float32)
    consts = _host_consts()
    nc = _build_nc()
    res = run_bass_kernel_spmd(nc, _in_maps(params, consts),
                               list(range(NCORES)), trace=True, tmpdir=tmpdir)
    return res.exec_time_ns
